# revision 2
# baseline (speedup 1.0000x reference)
"""Trainium2 Bass kernel for nn_MirrorResonance.

Math summary
------------
reference: H = tanh(X @ W1.T + b1); E = H @ W2.T + b2; o = wrap(E)
           phases: p <- mod(p + base + K*sin(o_t - p), 2pi)  over T=16384 steps
           out[s] = cos(phases + (s+1)*base) @ dec_W.T + dec_b

Design (all phase math in fp32 TURN units, z-transformed z_j = p_j - j*base):
 * Truncation: the scan contracts (~0.971/step); only the last W=576 steps
   matter (validated: 2.4e-4 rel err of exact truncated scan).
 * Encoder: hybrid precision. X/W1 split into bf16 hi+lo; early window
   columns (noise contracted away) use 1 product (hi*hi), late columns use 3
   (hi*hi + lo*hi + hi*lo) which restores ~fp32 accuracy. Same for H@W2.
 * Parallel-in-time scan: split W into M=16 chunks of L=36 steps, one
   partition group (8 attractors) per chunk, and run each chunk's scan from
   G=128 grid initial conditions living in the free axis. All chunks advance
   in lockstep: L serial steps instead of W.
   Step (d-state = wrapped angle diff, turns): s=sin(2pi d);
   q = d - (K/2pi)s; d' = wrap(q + delta_j) via round-to-int32 trick.
 * Each chunk yields its end-map F_c on the grid. Maps are unwrapped along
   the grid axis with one hardware scan instruction (cumsum of wrapped
   neighbor diffs), converted to deviation-from-identity psi in grid units.
 * Stitch: 16 sequential tent-weight interpolations (|.|/relu/fused
   multiply-reduce) compose the chunk maps at the running boundary estimate.
 * Generation: rank-17 matmul with host-precomputed cos/sin(t*base) rows,
   sharded over the 8 cores (128 output rows each).

Full-pipeline numpy simulation of this exact algorithm: rel err 8.8e-4.
"""

import numpy as np

TWO_PI = 2.0 * np.pi
DT = 0.01
K = 0.5
NCORES = 8
T_FULL = 16384
D = 1024
A = 8
S_OUT = 1024

W = 576          # scan window
M = 16           # chunks
L = W // M       # serial steps (36)
G = 128          # grid points per chunk map
SPLIT = 288      # window column where high-precision encoder starts
CH = 288         # encoder column chunk (psum-bank sized)

_cache = {}


# ---------------------------------------------------------------------------
# BIR legalization: this walrus build supports at most ONE sync-wait per
# instruction; split extra waits into single-wait EventSemaphore predecessors
# on the same engine (semantics preserved: engine stalls before the op).
# ---------------------------------------------------------------------------
def _install_birfix():
    if _cache.get("birfix"):
        return
    import orjson
    import concourse.bass_utils as bu
    import concourse.bass2jax as b2j

    orig = bu.compile_bir_kernel

    def _legalize(bir: bytes) -> bytes:
        d = orjson.loads(bir)
        for fn in d.get("functions", []):
            for blk in fn.get("blocks", []):
                out = []
                for inst in blk.get("instructions", []):
                    si = inst.get("sync_info") or {}
                    waits = si.get("on_wait") or []
                    if len(waits) > 1:
                        for k, w in enumerate(waits[:-1]):
                            out.append({
                                "debug": inst.get("debug", 0),
                                "engine": inst["engine"],
                                "ins": [], "outs": [],
                                "name": f"{inst['name']}_w{k}",
                                "opcode": "EventSemaphore",
                                "sync_info": {"on_update": [], "on_wait": [w]},
                            })
                        si["on_wait"] = [waits[-1]]
                    out.append(inst)
                blk["instructions"] = out
        return orjson.dumps(d)

    def wrapped(bir_json: bytes, tmpdir: str, neff_name="file.neff"):
        return orig(_legalize(bir_json), tmpdir, neff_name)

    bu.compile_bir_kernel = wrapped
    b2j.compile_bir_kernel = wrapped
    _cache["birfix"] = True


def _build_nc():
    import concourse.bass as bass
    import concourse.tile as tile
    import concourse.mybir as mybir
    from concourse.alu_op_type import AluOpType as OP

    F32 = mybir.dt.float32
    I32 = mybir.dt.int32
    BF16 = mybir.dt.bfloat16
    AF = mybir.ActivationFunctionType
    INV_2PI = float(1.0 / TWO_PI)
    KT = float(K / TWO_PI)
    HALF_PI = float(np.pi / 2.0)
    GF = float(G)
    LATE = W - SPLIT  # columns in the high-precision range

    nc = bass.Bass("TRN2")
    xhiT = nc.dram_tensor("xhiT", [D, W], BF16, kind="ExternalInput")
    xloT = nc.dram_tensor("xloT", [D, LATE], BF16, kind="ExternalInput")
    w1hiT = nc.dram_tensor("w1hiT", [D, D], BF16, kind="ExternalInput")
    w1loT = nc.dram_tensor("w1loT", [D, D], BF16, kind="ExternalInput")
    w2hiT = nc.dram_tensor("w2hiT", [D, A], BF16, kind="ExternalInput")
    w2loT = nc.dram_tensor("w2loT", [D, A], BF16, kind="ExternalInput")
    b1g = nc.dram_tensor("b1g", [128, 8], F32, kind="ExternalInput")
    obT_d = nc.dram_tensor("obT", [A, W], F32, kind="ExternalInput")
    iotaZ0_d = nc.dram_tensor("iotaZ0", [128, G], F32, kind="ExternalInput")
    iotaV_d = nc.dram_tensor("iotaV", [128, G], F32, kind="ExternalInput")
    iotaU_d = nc.dram_tensor("iotaU", [A, G + 1], F32, kind="ExternalInput")
    decwt = nc.dram_tensor("decwt", [A, D], F32, kind="ExternalInput")
    decb = nc.dram_tensor("decb", [1, D], F32, kind="ExternalInput")
    cs = nc.dram_tensor("cs", [17, 128], F32, kind="ExternalInput")
    out = nc.dram_tensor("out", [128, D], F32, kind="ExternalOutput")

    with tile.TileContext(nc) as tc:
        with (
            tc.tile_pool(name="sb", bufs=1) as sb,
            tc.tile_pool(name="ps", bufs=2, space="PSUM") as ps,
        ):
            w1h = [sb.tile([128, D], BF16, name=f"w1h{k}", tag=f"w1h{k}") for k in range(8)]
            w1l = [sb.tile([128, D], BF16, name=f"w1l{k}", tag=f"w1l{k}") for k in range(8)]
            xh = [sb.tile([128, W], BF16, name=f"xh{k}", tag=f"xh{k}") for k in range(8)]
            xl = [sb.tile([128, LATE], BF16, name=f"xl{k}", tag=f"xl{k}") for k in range(8)]
            w2h = sb.tile([128, 8, A], BF16)
            w2l = sb.tile([128, 8, A], BF16)
            b1sb = sb.tile([128, 8], F32)
            obT = sb.tile([A, W], F32)
            hf = sb.tile([128, CH], F32)
            hhi = sb.tile([128, CH], BF16)
            hlo = sb.tile([128, CH], BF16)
            e2 = sb.tile([A, W], F32)
            rE = sb.tile([A, W], I32)
            oTt = sb.tile([A, W], F32)
            oR = sb.tile([128, L], F32)
            dd = sb.tile([128, L - 1], F32)
            rdd = sb.tile([128, L - 1], I32)
            dR = sb.tile([128, L - 1], F32)
            iotaZ0 = sb.tile([128, G], F32)
            iotaV = sb.tile([128, G], F32)
            iotaU = sb.tile([A, G + 1], F32)
            dst = sb.tile([128, G], F32)
            s_t = sb.tile([128, G], F32)
            q_t = sb.tile([128, G], F32)
            qq_t = sb.tile([128, G], F32)
            r_t = sb.tile([128, G], I32)
            zL = sb.tile([128, G], F32)
            wdt = sb.tile([128, G - 1], F32)
            rwd = sb.tile([128, G - 1], I32)
            ones_t = sb.tile([128, G - 1], F32)
            zu = sb.tile([128, G], F32)
            psig = sb.tile([128, G + 1], F32)
            w1e = sb.tile([128, 1], F32)
            rex = sb.tile([128, 1], I32)
            w2e = sb.tile([128, 1], F32)
            zue = sb.tile([128, 1], F32)
            zmapS = sb.tile([A, M * (G + 1)], F32)
            xst = [sb.tile([A, 1], F32, name=f"xst{i}") for i in range(2)]
            rg_t = sb.tile([A, 1], I32)
            xf_t = sb.tile([A, 1], F32)
            dab_t = sb.tile([A, G + 1], F32)
            wt_t = sb.tile([A, G + 1], F32)
            scr_t = sb.tile([A, G + 1], F32)
            y_t = sb.tile([A, 1], F32)
            one8 = sb.tile([A, 1], F32)
            cpih = sb.tile([A, 1], F32)
            zt_t = sb.tile([A, 1], F32)
            rT_t = sb.tile([A, 1], I32)
            zw_t = sb.tile([A, 1], F32)
            az_t = sb.tile([A, 1], F32)
            uc_t = sb.tile([A, 1], F32)
            vs_t = sb.tile([A, 1], F32)
            dwsb = sb.tile([A, D], F32)
            r_u = sb.tile([A, D], F32)
            r_v = sb.tile([A, D], F32)
            dbsb = sb.tile([1, D], F32)
            csu = sb.tile([A, 128], F32)
            csv = sb.tile([A, 128], F32)
            cs1 = sb.tile([1, 128], F32)
            outsb = sb.tile([128, D], F32)

            dma = nc.sync
            for k in range(8):
                dma.dma_start(xh[k][:], xhiT[k * 128:(k + 1) * 128, :])
                dma.dma_start(w1h[k][:], w1hiT[k * 128:(k + 1) * 128, :])
            for k in range(8):
                dma.dma_start(xl[k][:], xloT[k * 128:(k + 1) * 128, :])
                dma.dma_start(w1l[k][:], w1loT[k * 128:(k + 1) * 128, :])
                dma.dma_start(w2h[:, k, :], w2hiT[k * 128:(k + 1) * 128, :])
                dma.dma_start(w2l[:, k, :], w2loT[k * 128:(k + 1) * 128, :])
            dma.dma_start(b1sb[:], b1g[:])
            dma.dma_start(obT[:], obT_d[:])
            dma.dma_start(iotaZ0[:], iotaZ0_d[:])
            dma.dma_start(iotaV[:], iotaV_d[:])
            dma.dma_start(iotaU[:], iotaU_d[:])
            dma.dma_start(dwsb[:], decwt[:])
            dma.dma_start(dbsb[:], decb[:])
            dma.dma_start(csu[:], cs[0:8, :])
            dma.dma_start(csv[:], cs[8:16, :])
            dma.dma_start(cs1[:], cs[16:17, :])
            nc.vector.memset(ones_t[:], 1.0)
            nc.vector.memset(one8[:], 1.0)
            nc.vector.memset(cpih[:], HALF_PI)
            nc.vector.memset(xst[0][:], 0.0)

            # ---------------- encoder ----------------
            with nc.named_scope("encoder"):
                for ch, (c0, c1) in enumerate(((0, SPLIT), (SPLIT, W))):
                    wc = c1 - c0
                    late = ch == 1
                    pe = ps.tile([A, CH], F32, tag=f"pe{ch}")
                    for nt in range(8):
                        nsl = slice(nt * 128, (nt + 1) * 128)
                        ph = ps.tile([128, CH], F32, tag="ph")
                        for kt in range(8):
                            last = kt == 7
                            nc.tensor.matmul(ph[:, 0:wc], w1h[kt][:, nsl],
                                             xh[kt][:, c0:c1],
                                             start=(kt == 0),
                                             stop=(last and not late))
                            if late:
                                nc.tensor.matmul(ph[:, 0:wc], w1h[kt][:, nsl],
                                                 xl[kt][:, 0:wc],
                                                 start=False, stop=False)
                                nc.tensor.matmul(ph[:, 0:wc], w1l[kt][:, nsl],
                                                 xh[kt][:, c0:c1],
                                                 start=False, stop=last)
                        if not late:
                            nc.scalar.activation(hhi[:, 0:wc], ph[:, 0:wc], AF.Tanh,
                                                 bias=b1sb[:, nt:nt + 1], scale=1.0)
                            nc.tensor.matmul(pe[:, 0:wc], w2h[:, nt, :], hhi[:, 0:wc],
                                             start=(nt == 0), stop=(nt == 7))
                        else:
                            nc.scalar.activation(hf[:, 0:wc], ph[:, 0:wc], AF.Tanh,
                                                 bias=b1sb[:, nt:nt + 1], scale=1.0)
                            nc.vector.tensor_scalar(hhi[:, 0:wc], hf[:, 0:wc],
                                                    1.0, 0.0, OP.mult, OP.add)
                            nc.vector.scalar_tensor_tensor(hlo[:, 0:wc], hhi[:, 0:wc],
                                                           -1.0, hf[:, 0:wc],
                                                           OP.mult, OP.add)
                            nc.tensor.matmul(pe[:, 0:wc], w2h[:, nt, :], hhi[:, 0:wc],
                                             start=(nt == 0), stop=False)
                            nc.tensor.matmul(pe[:, 0:wc], w2h[:, nt, :], hlo[:, 0:wc],
                                             start=False, stop=False)
                            nc.tensor.matmul(pe[:, 0:wc], w2l[:, nt, :], hhi[:, 0:wc],
                                             start=False, stop=(nt == 7))
                    # e2 = E/2pi + obT  (turns, unwrapped)
                    nc.vector.scalar_tensor_tensor(e2[:, c0:c1], pe[:, 0:wc],
                                                   INV_2PI, obT[:, c0:c1],
                                                   OP.mult, OP.add)

            # ---------------- obs wrap + rearrange ----------------
            with nc.named_scope("osb"):
                nc.vector.tensor_scalar(rE[:], e2[:], 1.0, 0.0, OP.mult, OP.add)
                nc.vector.scalar_tensor_tensor(oTt[:], rE[:], -1.0, e2[:],
                                               OP.mult, OP.add)
                for c in range(M):
                    dma.dma_start(oR[c * 8:(c + 1) * 8, :],
                                  oTt[:, c * L:(c + 1) * L])
                nc.vector.tensor_tensor(dd[:], oR[:, 1:L], oR[:, 0:L - 1],
                                        OP.subtract)
                nc.vector.tensor_scalar(rdd[:], dd[:], 1.0, 0.0, OP.mult, OP.add)
                nc.vector.scalar_tensor_tensor(dR[:], rdd[:], -1.0, dd[:],
                                               OP.mult, OP.add)

            # ---------------- grid scan ----------------
            with nc.named_scope("scan"):
                # d0 = wrap(oR[:,0] - z0grid)
                nc.vector.tensor_scalar(q_t[:], iotaZ0[:], -1.0, oR[:, 0:1],
                                        OP.mult, OP.add)
                nc.vector.tensor_scalar(r_t[:], q_t[:], 1.0, 0.0, OP.mult, OP.add)
                nc.vector.scalar_tensor_tensor(dst[:], r_t[:], -1.0, q_t[:],
                                               OP.mult, OP.add)
                for j in range(L - 1):
                    nc.scalar.activation(s_t[:], dst[:], AF.Sin,
                                         bias=0.0, scale=TWO_PI)
                    nc.vector.scalar_tensor_tensor(q_t[:], s_t[:], -KT, dst[:],
                                                   OP.mult, OP.add)
                    nc.vector.tensor_scalar(qq_t[:], q_t[:], dR[:, j:j + 1], 0.0,
                                            OP.add, OP.add)
                    nc.vector.tensor_scalar(r_t[:], qq_t[:], 1.0, 0.0,
                                            OP.mult, OP.add)
                    nc.vector.scalar_tensor_tensor(dst[:], r_t[:], -1.0, qq_t[:],
                                                   OP.mult, OP.add)
                # final partial step -> map values zL
                nc.scalar.activation(s_t[:], dst[:], AF.Sin, bias=0.0, scale=TWO_PI)
                nc.vector.scalar_tensor_tensor(q_t[:], s_t[:], -KT, dst[:],
                                               OP.mult, OP.add)
                nc.vector.tensor_scalar(zL[:], q_t[:], -1.0, oR[:, L - 1:L],
                                        OP.mult, OP.add)

            # ---------------- unwrap maps + psi ----------------
            with nc.named_scope("maps"):
                nc.vector.tensor_tensor(wdt[:], zL[:, 1:G], zL[:, 0:G - 1],
                                        OP.subtract)
                nc.vector.tensor_scalar(rwd[:], wdt[:], 1.0, 0.0, OP.mult, OP.add)
                nc.vector.scalar_tensor_tensor(wdt[:], rwd[:], -1.0, wdt[:],
                                               OP.mult, OP.add)
                nc.vector.tensor_copy(zu[:, 0:1], zL[:, 0:1])
                nc.vector.tensor_tensor_scan(zu[:, 1:G], ones_t[:], wdt[:],
                                             zL[:, 0:1], OP.mult, OP.add)
                nc.vector.scalar_tensor_tensor(psig[:, 0:G], zu[:], GF, iotaV[:],
                                               OP.mult, OP.subtract)
                # extension column (grid point +64)
                nc.vector.tensor_tensor(w1e[:], zL[:, 0:1], zL[:, G - 1:G],
                                        OP.subtract)
                nc.vector.tensor_scalar(rex[:], w1e[:], 1.0, 0.0, OP.mult, OP.add)
                nc.vector.scalar_tensor_tensor(w2e[:], rex[:], -1.0, w1e[:],
                                               OP.mult, OP.add)
                nc.vector.tensor_tensor(zue[:], zu[:, G - 1:G], w2e[:], OP.add)
                nc.vector.tensor_scalar(psig[:, G:G + 1], zue[:], GF, 64.0,
                                        OP.mult, OP.subtract)
                for c in range(M):
                    dma.dma_start(zmapS[:, c * (G + 1):(c + 1) * (G + 1)],
                                  psig[c * 8:(c + 1) * 8, :])

            # ---------------- stitch ----------------
            with nc.named_scope("stitch"):
                for c in range(M):
                    xin = xst[c % 2][:]
                    xout = xst[(c + 1) % 2][:]
                    nc.vector.tensor_scalar(rg_t[:], xin, float(1.0 / G), 0.0,
                                            OP.mult, OP.add)
                    nc.vector.scalar_tensor_tensor(xf_t[:], rg_t[:], -GF, xin,
                                                   OP.mult, OP.add)
                    nc.scalar.activation(dab_t[:], iotaU[:], AF.Abs,
                                         bias=xf_t[:], scale=-1.0)
                    nc.scalar.activation(wt_t[:], dab_t[:], AF.Relu,
                                         bias=one8[:], scale=-1.0)
                    nc.vector.scalar_tensor_tensor(
                        scr_t[:], wt_t[:], 1.0,
                        zmapS[:, c * (G + 1):(c + 1) * (G + 1)],
                        OP.mult, OP.mult, accum_out=y_t[:])
                    nc.vector.tensor_tensor(xout, y_t[:], xin, OP.add)

            # ---------------- tail: generation ----------------
            with nc.named_scope("tail"):
                xfin = xst[M % 2][:]
                nc.vector.tensor_scalar(zt_t[:], xfin, float(1.0 / G), 0.0,
                                        OP.mult, OP.add)
                nc.vector.tensor_scalar(rT_t[:], zt_t[:], 1.0, 0.0, OP.mult, OP.add)
                nc.vector.scalar_tensor_tensor(zw_t[:], rT_t[:], -1.0, zt_t[:],
                                               OP.mult, OP.add)
                nc.scalar.activation(az_t[:], zw_t[:], AF.Abs, bias=0.0, scale=1.0)
                nc.scalar.activation(uc_t[:], az_t[:], AF.Sin,
                                     bias=cpih[:], scale=-TWO_PI)   # cos(2pi z)
                nc.scalar.activation(vs_t[:], zw_t[:], AF.Sin,
                                     bias=0.0, scale=TWO_PI)        # sin(2pi z)
                nc.scalar.activation(r_u[:], dwsb[:], AF.Copy,
                                     bias=0.0, scale=uc_t[:])
                nc.vector.tensor_scalar(r_v[:], dwsb[:], vs_t[:], 0.0,
                                        OP.mult, OP.add)
                for half in range(2):
                    hs = slice(half * 512, (half + 1) * 512)
                    po = ps.tile([128, 512], F32, tag="po")
                    nc.tensor.matmul(po[:], csu[:], r_u[:, hs],
                                     start=True, stop=False)
                    nc.tensor.matmul(po[:], csv[:], r_v[:, hs],
                                     start=False, stop=False)
                    nc.tensor.matmul(po[:], cs1[:], dbsb[:, hs],
                                     start=False, stop=True)
                    nc.vector.tensor_copy(outsb[:, hs], po[:])
                dma.dma_start(out[:], outsb[:])

    return nc


def kernel(**inputs) -> np.ndarray:
    _install_birfix()
    from concourse.bass_utils import run_bass_kernel_spmd
    import ml_dtypes

    bf16 = ml_dtypes.bfloat16

    X = np.asarray(inputs["observed_trajectory"], dtype=np.float32)
    W1 = np.asarray(inputs["W1"], dtype=np.float32)
    b1 = np.asarray(inputs["b1"], dtype=np.float32)
    W2 = np.asarray(inputs["W2"], dtype=np.float32)
    b2 = np.asarray(inputs["b2"], dtype=np.float64)
    freqs = np.asarray(inputs["freqs"], dtype=np.float64)
    dec_W = np.asarray(inputs["dec_W"], dtype=np.float32)
    dec_b = np.asarray(inputs["dec_b"], dtype=np.float32)
    num_steps = int(np.asarray(inputs["num_steps"]))
    T, D_ = X.shape
    assert (T, D_, num_steps) == (T_FULL, D, S_OUT), (T, D_, num_steps)

    base = freqs * TWO_PI * DT            # (A,) rad/step
    baseT = base / TWO_PI                 # turns/step

    t0 = T - W
    Xw = X[t0:]                            # (W, D)
    xT = np.ascontiguousarray(Xw.T)        # (D, W) fp32
    xhi = xT.astype(bf16)
    xlo = (xT - xhi.astype(np.float32)).astype(bf16)
    w1T = np.ascontiguousarray(W1.T)       # (D, D)
    w1hi = w1T.astype(bf16)
    w1lo = (w1T - w1hi.astype(np.float32)).astype(bf16)
    w2T = np.ascontiguousarray(W2.T)       # (D, A)
    w2hi = w2T.astype(bf16)
    w2lo = (w2T - w2hi.astype(np.float32)).astype(bf16)
    b1g = np.ascontiguousarray(b1.reshape(8, 128).T)

    j = np.arange(W, dtype=np.float64)
    ob = b2[:, None] / TWO_PI - j[None, :] * baseT[:, None]
    obT = (ob - np.round(ob)).astype(np.float32)        # (A, W) wrapped turns

    gv = np.arange(G, dtype=np.float32) - 64.0
    iotaZ0 = np.broadcast_to(gv / G, (128, G)).astype(np.float32).copy()
    iotaV = np.broadcast_to(gv, (128, G)).astype(np.float32).copy()
    iotaU = np.broadcast_to(np.arange(G + 1, dtype=np.float32) - 64.0,
                            (A, G + 1)).astype(np.float32).copy()

    decwt = np.ascontiguousarray(dec_W.T)
    decb = np.ascontiguousarray(dec_b.reshape(1, D))

    in_maps = []
    rows = S_OUT // NCORES
    for c in range(NCORES):
        s = np.arange(c * rows, (c + 1) * rows, dtype=np.float64)
        th = TWO_PI * ((W + s[None, :] + 1.0) * baseT[:, None])   # (A, rows)
        csm = np.empty((17, rows), np.float32)
        csm[0:8] = np.cos(th)
        csm[8:16] = -np.sin(th)
        csm[16] = 1.0
        in_maps.append({
            "xhiT": xhi, "xloT": np.ascontiguousarray(xlo[:, SPLIT:]),
            "w1hiT": w1hi, "w1loT": w1lo,
            "w2hiT": w2hi, "w2loT": w2lo,
            "b1g": b1g, "obT": obT,
            "iotaZ0": iotaZ0, "iotaV": iotaV, "iotaU": iotaU,
            "decwt": decwt, "decb": decb,
            "cs": np.ascontiguousarray(csm),
        })

    if "nc" not in _cache:
        _cache["nc"] = _build_nc()
    res = run_bass_kernel_spmd(_cache["nc"], in_maps, core_ids=list(range(NCORES)))
    out = np.concatenate([r["out"] for r in res.results], axis=0)
    return out.astype(np.float32)


# revision 8
# speedup vs baseline: 1.0818x; 1.0818x over previous
"""Trainium2 Bass kernel for nn_MirrorResonance.

Math summary
------------
reference: H = tanh(X @ W1.T + b1); E = H @ W2.T + b2; o = wrap(E)
           phases: p <- mod(p + base + K*sin(o_t - p), 2pi)  over T=16384 steps
           out[s] = cos(phases + (s+1)*base) @ dec_W.T + dec_b

Design (all phase math in fp32 TURN units, z-transformed z_j = p_j - j*base):
 * Truncation: the scan contracts (~0.971/step); only the last W=576 steps
   matter (validated: 2.4e-4 rel err of exact truncated scan).
 * Encoder: hybrid precision. X/W1 split into bf16 hi+lo; early window
   columns (noise contracted away) use 1 product (hi*hi), late columns use 3
   (hi*hi + lo*hi + hi*lo) which restores ~fp32 accuracy. Same for H@W2.
 * Parallel-in-time scan: split W into M=16 chunks of L=36 steps, one
   partition group (8 attractors) per chunk, and run each chunk's scan from
   G=128 grid initial conditions living in the free axis. All chunks advance
   in lockstep: L serial steps instead of W.
   Step (d-state = wrapped angle diff, turns): s=sin(2pi d);
   q = d - (K/2pi)s; d' = wrap(q + delta_j) via round-to-int32 trick.
 * Each chunk yields its end-map F_c on the grid. Maps are unwrapped along
   the grid axis with one hardware scan instruction (cumsum of wrapped
   neighbor diffs), converted to deviation-from-identity psi in grid units.
 * Stitch: 16 sequential tent-weight interpolations (|.|/relu/fused
   multiply-reduce) compose the chunk maps at the running boundary estimate.
 * Generation: rank-17 matmul with host-precomputed cos/sin(t*base) rows,
   sharded over the 8 cores (128 output rows each).

Full-pipeline numpy simulation of this exact algorithm: rel err 8.8e-4.
"""

import numpy as np

TWO_PI = 2.0 * np.pi
DT = 0.01
K = 0.5
NCORES = 8
T_FULL = 16384
D = 1024
A = 8
S_OUT = 1024

W = 576          # scan window
M = 16           # chunks
L = W // M       # serial steps (36)
G = 128          # grid points per chunk map
SPLIT = 288      # window column where high-precision encoder starts
CH = 288         # encoder column chunk (psum-bank sized)

_cache = {}


# ---------------------------------------------------------------------------
# BIR legalization: this walrus build supports at most ONE sync-wait per
# instruction; split extra waits into single-wait EventSemaphore predecessors
# on the same engine (semantics preserved: engine stalls before the op).
# ---------------------------------------------------------------------------
def _install_birfix():
    if _cache.get("birfix"):
        return
    import orjson
    import concourse.bass_utils as bu
    import concourse.bass2jax as b2j

    orig = bu.compile_bir_kernel

    def _legalize(bir: bytes) -> bytes:
        d = orjson.loads(bir)
        for fn in d.get("functions", []):
            for blk in fn.get("blocks", []):
                out = []
                for inst in blk.get("instructions", []):
                    si = inst.get("sync_info") or {}
                    waits = si.get("on_wait") or []
                    if len(waits) > 1:
                        for k, w in enumerate(waits[:-1]):
                            out.append({
                                "debug": inst.get("debug", 0),
                                "engine": inst["engine"],
                                "ins": [], "outs": [],
                                "name": f"{inst['name']}_w{k}",
                                "opcode": "EventSemaphore",
                                "sync_info": {"on_update": [], "on_wait": [w]},
                            })
                        si["on_wait"] = [waits[-1]]
                    out.append(inst)
                blk["instructions"] = out
        return orjson.dumps(d)

    def wrapped(bir_json: bytes, tmpdir: str, neff_name="file.neff"):
        return orig(_legalize(bir_json), tmpdir, neff_name)

    bu.compile_bir_kernel = wrapped
    b2j.compile_bir_kernel = wrapped
    _cache["birfix"] = True


def _build_nc():
    import concourse.bass as bass
    import concourse.tile as tile
    import concourse.mybir as mybir
    from concourse.alu_op_type import AluOpType as OP

    F32 = mybir.dt.float32
    I32 = mybir.dt.int32
    BF16 = mybir.dt.bfloat16
    AF = mybir.ActivationFunctionType
    INV_2PI = float(1.0 / TWO_PI)
    KT = float(K / TWO_PI)
    HALF_PI = float(np.pi / 2.0)
    GF = float(G)
    LATE = W - SPLIT  # columns in the high-precision range

    nc = bass.Bass("TRN2")
    xhiT = nc.dram_tensor("xhiT", [D, W], BF16, kind="ExternalInput")
    xloT = nc.dram_tensor("xloT", [D, LATE], BF16, kind="ExternalInput")
    w1hiT = nc.dram_tensor("w1hiT", [D, D], BF16, kind="ExternalInput")
    w1loT = nc.dram_tensor("w1loT", [D, D], BF16, kind="ExternalInput")
    w2hiT = nc.dram_tensor("w2hiT", [D, A], BF16, kind="ExternalInput")
    w2loT = nc.dram_tensor("w2loT", [D, A], BF16, kind="ExternalInput")
    b1g = nc.dram_tensor("b1g", [128, 8], F32, kind="ExternalInput")
    obT_d = nc.dram_tensor("obT", [A, W], F32, kind="ExternalInput")
    iotaZ0_d = nc.dram_tensor("iotaZ0", [128, G], F32, kind="ExternalInput")
    iotaV_d = nc.dram_tensor("iotaV", [128, G], F32, kind="ExternalInput")
    iotaU_d = nc.dram_tensor("iotaU", [A, G + 1], F32, kind="ExternalInput")
    decwt = nc.dram_tensor("decwt", [A, D], F32, kind="ExternalInput")
    decb = nc.dram_tensor("decb", [1, D], F32, kind="ExternalInput")
    cs = nc.dram_tensor("cs", [17, 128], F32, kind="ExternalInput")
    out = nc.dram_tensor("out", [128, D], F32, kind="ExternalOutput")

    with tile.TileContext(nc) as tc:
        with (
            tc.tile_pool(name="sb", bufs=1) as sb,
            tc.tile_pool(name="ps", bufs=2, space="PSUM") as ps,
        ):
            w1h = [sb.tile([128, D], BF16, name=f"w1h{k}", tag=f"w1h{k}") for k in range(8)]
            w1l = [sb.tile([128, D], BF16, name=f"w1l{k}", tag=f"w1l{k}") for k in range(8)]
            xh = [sb.tile([128, W], BF16, name=f"xh{k}", tag=f"xh{k}") for k in range(8)]
            xl = [sb.tile([128, LATE], BF16, name=f"xl{k}", tag=f"xl{k}") for k in range(8)]
            w2h = sb.tile([128, 8, A], BF16)
            w2l = sb.tile([128, 8, A], BF16)
            b1sb = sb.tile([128, 8], F32)
            obT = sb.tile([A, W], F32)
            hf = sb.tile([128, CH], F32)
            hhi = [sb.tile([128, CH], BF16, name=f"hhi{n}", tag=f"hhi{n}") for n in range(8)]
            hlo = [sb.tile([128, CH], BF16, name=f"hlo{n}", tag=f"hlo{n}") for n in range(8)]
            e2 = sb.tile([A, W], F32)
            rE = sb.tile([A, W], I32)
            oTt = sb.tile([A, W], F32)
            oR = sb.tile([128, L], F32)
            dd = sb.tile([128, L - 1], F32)
            rdd = sb.tile([128, L - 1], I32)
            dR = sb.tile([128, L - 1], F32)
            iotaZ0 = sb.tile([128, G], F32)
            iotaV = sb.tile([128, G], F32)
            iotaU = sb.tile([A, G + 1], F32)
            dst = sb.tile([128, G], F32)
            s_t = sb.tile([128, G], F32)
            q_t = sb.tile([128, G], F32)
            qq_t = sb.tile([128, G], F32)
            r_t = sb.tile([128, G], I32)
            zL = sb.tile([128, G], F32)
            wdt = sb.tile([128, G - 1], F32)
            rwd = sb.tile([128, G - 1], I32)
            ones_t = sb.tile([128, G - 1], F32)
            zu = sb.tile([128, G], F32)
            psig = sb.tile([128, G + 1], F32)
            w1e = sb.tile([128, 1], F32)
            rex = sb.tile([128, 1], I32)
            w2e = sb.tile([128, 1], F32)
            zue = sb.tile([128, 1], F32)
            zmapS = sb.tile([A, M * (G + 1)], F32)
            xst = [sb.tile([A, 1], F32, name=f"xst{i}") for i in range(2)]
            rg_t = sb.tile([A, 1], I32)
            xf_t = sb.tile([A, 1], F32)
            dab_t = sb.tile([A, G + 1], F32)
            wt_t = sb.tile([A, G + 1], F32)
            scr_t = sb.tile([A, G + 1], F32)
            y_t = sb.tile([A, 1], F32)
            one8 = sb.tile([A, 1], F32)
            cpih = sb.tile([A, 1], F32)
            zt_t = sb.tile([A, 1], F32)
            rT_t = sb.tile([A, 1], I32)
            zw_t = sb.tile([A, 1], F32)
            az_t = sb.tile([A, 1], F32)
            uc_t = sb.tile([A, 1], F32)
            vs_t = sb.tile([A, 1], F32)
            dwsb = sb.tile([A, D], F32)
            r_u = sb.tile([A, D], F32)
            r_v = sb.tile([A, D], F32)
            dbsb = sb.tile([1, D], F32)
            csu = sb.tile([A, 128], F32)
            csv = sb.tile([A, 128], F32)
            cs1 = sb.tile([1, 128], F32)
            outsb = sb.tile([128, D], F32)

            dma = nc.sync
            for k in range(8):
                dma.dma_start(xh[k][:], xhiT[k * 128:(k + 1) * 128, :])
                dma.dma_start(w1h[k][:], w1hiT[k * 128:(k + 1) * 128, :])
            for k in range(8):
                dma.dma_start(xl[k][:], xloT[k * 128:(k + 1) * 128, :])
                dma.dma_start(w1l[k][:], w1loT[k * 128:(k + 1) * 128, :])
                dma.dma_start(w2h[:, k, :], w2hiT[k * 128:(k + 1) * 128, :])
                dma.dma_start(w2l[:, k, :], w2loT[k * 128:(k + 1) * 128, :])
            dma.dma_start(b1sb[:], b1g[:])
            dma.dma_start(obT[:], obT_d[:])
            dma.dma_start(iotaZ0[:], iotaZ0_d[:])
            dma.dma_start(iotaV[:], iotaV_d[:])
            dma.dma_start(iotaU[:], iotaU_d[:])
            dma.dma_start(dwsb[:], decwt[:])
            dma.dma_start(dbsb[:], decb[:])
            dma.dma_start(csu[:], cs[0:8, :])
            dma.dma_start(csv[:], cs[8:16, :])
            dma.dma_start(cs1[:], cs[16:17, :])
            nc.vector.memset(ones_t[:], 1.0)
            nc.vector.memset(one8[:], 1.0)
            nc.vector.memset(cpih[:], HALF_PI)
            nc.vector.memset(xst[0][:], 0.0)

            # ---------------- encoder ----------------
            with nc.named_scope("encoder"):
                for ch, (c0, c1) in enumerate(((0, SPLIT), (SPLIT, W))):
                    wc = c1 - c0
                    late = ch == 1
                    pe = ps.tile([A, CH], F32, tag=f"pe{ch}")
                    for nt in range(8):
                        nsl = slice(nt * 128, (nt + 1) * 128)
                        ph = ps.tile([128, CH], F32, tag="ph")
                        for kt in range(8):
                            last = kt == 7
                            nc.tensor.matmul(ph[:, 0:wc], w1h[kt][:, nsl],
                                             xh[kt][:, c0:c1],
                                             start=(kt == 0),
                                             stop=(last and not late))
                            if late:
                                nc.tensor.matmul(ph[:, 0:wc], w1h[kt][:, nsl],
                                                 xl[kt][:, 0:wc],
                                                 start=False, stop=False)
                                nc.tensor.matmul(ph[:, 0:wc], w1l[kt][:, nsl],
                                                 xh[kt][:, c0:c1],
                                                 start=False, stop=last)
                        if not late:
                            nc.scalar.activation(hhi[nt][:, 0:wc], ph[:, 0:wc],
                                                 AF.Tanh,
                                                 bias=b1sb[:, nt:nt + 1], scale=1.0)
                        else:
                            nc.scalar.activation(hf[:, 0:wc], ph[:, 0:wc], AF.Tanh,
                                                 bias=b1sb[:, nt:nt + 1], scale=1.0)
                            nc.vector.tensor_scalar(hhi[nt][:, 0:wc], hf[:, 0:wc],
                                                    1.0, 0.0, OP.mult, OP.add)
                            nc.vector.scalar_tensor_tensor(hlo[nt][:, 0:wc],
                                                           hhi[nt][:, 0:wc],
                                                           -1.0, hf[:, 0:wc],
                                                           OP.mult, OP.add)
                    # E-GEMM after all H tiles: keeps the PE free of Act stalls
                    for nt in range(8):
                        if not late:
                            nc.tensor.matmul(pe[:, 0:wc], w2h[:, nt, :],
                                             hhi[nt][:, 0:wc],
                                             start=(nt == 0), stop=(nt == 7))
                        else:
                            nc.tensor.matmul(pe[:, 0:wc], w2h[:, nt, :],
                                             hhi[nt][:, 0:wc],
                                             start=(nt == 0), stop=False)
                            nc.tensor.matmul(pe[:, 0:wc], w2h[:, nt, :],
                                             hlo[nt][:, 0:wc],
                                             start=False, stop=False)
                            nc.tensor.matmul(pe[:, 0:wc], w2l[:, nt, :],
                                             hhi[nt][:, 0:wc],
                                             start=False, stop=(nt == 7))
                    # e2 = E/2pi + obT  (turns, unwrapped)
                    nc.vector.scalar_tensor_tensor(e2[:, c0:c1], pe[:, 0:wc],
                                                   INV_2PI, obT[:, c0:c1],
                                                   OP.mult, OP.add)

            # ---------------- obs wrap + rearrange ----------------
            with nc.named_scope("osb"):
                nc.vector.tensor_scalar(rE[:], e2[:], 1.0, 0.0, OP.mult, OP.add)
                nc.vector.scalar_tensor_tensor(oTt[:], rE[:], -1.0, e2[:],
                                               OP.mult, OP.add)
                qs = (nc.sync, nc.scalar, nc.gpsimd)
                for c in range(M):
                    qs[c % 3].dma_start(oR[c * 8:(c + 1) * 8, :],
                                        oTt[:, c * L:(c + 1) * L])
                nc.vector.tensor_tensor(dd[:], oR[:, 1:L], oR[:, 0:L - 1],
                                        OP.subtract)
                nc.vector.tensor_scalar(rdd[:], dd[:], 1.0, 0.0, OP.mult, OP.add)
                nc.vector.scalar_tensor_tensor(dR[:], rdd[:], -1.0, dd[:],
                                               OP.mult, OP.add)

            # ---------------- grid scan ----------------
            with nc.named_scope("scan"):
                # d0 = wrap(oR[:,0] - z0grid)
                nc.vector.tensor_scalar(q_t[:], iotaZ0[:], -1.0, oR[:, 0:1],
                                        OP.mult, OP.add)
                nc.vector.tensor_scalar(r_t[:], q_t[:], 1.0, 0.0, OP.mult, OP.add)
                nc.vector.scalar_tensor_tensor(dst[:], r_t[:], -1.0, q_t[:],
                                               OP.mult, OP.add)
                for j in range(L - 1):
                    nc.scalar.activation(s_t[:], dst[:], AF.Sin,
                                         bias=0.0, scale=TWO_PI)
                    nc.vector.scalar_tensor_tensor(q_t[:], s_t[:], -KT, dst[:],
                                                   OP.mult, OP.add)
                    # qq and its rounding both depend only on q: issue
                    # back-to-back so they pipeline on the DVE
                    nc.vector.tensor_scalar(qq_t[:], q_t[:], dR[:, j:j + 1], 0.0,
                                            OP.add, OP.add)
                    nc.vector.tensor_scalar(r_t[:], q_t[:], dR[:, j:j + 1], 0.0,
                                            OP.add, OP.add)
                    nc.vector.scalar_tensor_tensor(dst[:], r_t[:], -1.0, qq_t[:],
                                                   OP.mult, OP.add)
                # final partial step -> map values zL
                nc.scalar.activation(s_t[:], dst[:], AF.Sin, bias=0.0, scale=TWO_PI)
                nc.vector.scalar_tensor_tensor(q_t[:], s_t[:], -KT, dst[:],
                                               OP.mult, OP.add)
                nc.vector.tensor_scalar(zL[:], q_t[:], -1.0, oR[:, L - 1:L],
                                        OP.mult, OP.add)

            # ---------------- unwrap maps + psi ----------------
            with nc.named_scope("maps"):
                nc.vector.tensor_tensor(wdt[:], zL[:, 1:G], zL[:, 0:G - 1],
                                        OP.subtract)
                nc.vector.tensor_scalar(rwd[:], wdt[:], 1.0, 0.0, OP.mult, OP.add)
                nc.vector.scalar_tensor_tensor(wdt[:], rwd[:], -1.0, wdt[:],
                                               OP.mult, OP.add)
                nc.vector.tensor_copy(zu[:, 0:1], zL[:, 0:1])
                nc.vector.tensor_tensor_scan(zu[:, 1:G], ones_t[:], wdt[:],
                                             zL[:, 0:1], OP.mult, OP.add)
                nc.vector.scalar_tensor_tensor(psig[:, 0:G], zu[:], GF, iotaV[:],
                                               OP.mult, OP.subtract)
                # extension column (grid point +64)
                nc.vector.tensor_tensor(w1e[:], zL[:, 0:1], zL[:, G - 1:G],
                                        OP.subtract)
                nc.vector.tensor_scalar(rex[:], w1e[:], 1.0, 0.0, OP.mult, OP.add)
                nc.vector.scalar_tensor_tensor(w2e[:], rex[:], -1.0, w1e[:],
                                               OP.mult, OP.add)
                nc.vector.tensor_tensor(zue[:], zu[:, G - 1:G], w2e[:], OP.add)
                nc.vector.tensor_scalar(psig[:, G:G + 1], zue[:], GF, 64.0,
                                        OP.mult, OP.subtract)
                qs = (nc.sync, nc.scalar, nc.gpsimd)
                for c in range(M):
                    qs[c % 3].dma_start(zmapS[:, c * (G + 1):(c + 1) * (G + 1)],
                                        psig[c * 8:(c + 1) * 8, :])

            # ---------------- stitch ----------------
            with nc.named_scope("stitch"):
                for c in range(M):
                    xin = xst[c % 2][:]
                    xout = xst[(c + 1) % 2][:]
                    nc.vector.tensor_scalar(rg_t[:], xin, float(1.0 / G), 0.0,
                                            OP.mult, OP.add)
                    nc.vector.scalar_tensor_tensor(xf_t[:], rg_t[:], -GF, xin,
                                                   OP.mult, OP.add)
                    nc.scalar.activation(dab_t[:], iotaU[:], AF.Abs,
                                         bias=xf_t[:], scale=-1.0)
                    nc.scalar.activation(wt_t[:], dab_t[:], AF.Relu,
                                         bias=one8[:], scale=-1.0)
                    nc.vector.scalar_tensor_tensor(
                        scr_t[:], wt_t[:], 1.0,
                        zmapS[:, c * (G + 1):(c + 1) * (G + 1)],
                        OP.mult, OP.mult, accum_out=y_t[:])
                    nc.vector.tensor_tensor(xout, y_t[:], xin, OP.add)

            # ---------------- tail: generation ----------------
            with nc.named_scope("tail"):
                xfin = xst[M % 2][:]
                nc.vector.tensor_scalar(zt_t[:], xfin, float(1.0 / G), 0.0,
                                        OP.mult, OP.add)
                nc.vector.tensor_scalar(rT_t[:], zt_t[:], 1.0, 0.0, OP.mult, OP.add)
                nc.vector.scalar_tensor_tensor(zw_t[:], rT_t[:], -1.0, zt_t[:],
                                               OP.mult, OP.add)
                nc.scalar.activation(az_t[:], zw_t[:], AF.Abs, bias=0.0, scale=1.0)
                nc.scalar.activation(uc_t[:], az_t[:], AF.Sin,
                                     bias=cpih[:], scale=-TWO_PI)   # cos(2pi z)
                nc.scalar.activation(vs_t[:], zw_t[:], AF.Sin,
                                     bias=0.0, scale=TWO_PI)        # sin(2pi z)
                nc.scalar.activation(r_u[:], dwsb[:], AF.Copy,
                                     bias=0.0, scale=uc_t[:])
                nc.vector.tensor_scalar(r_v[:], dwsb[:], vs_t[:], 0.0,
                                        OP.mult, OP.add)
                for half in range(2):
                    hs = slice(half * 512, (half + 1) * 512)
                    po = ps.tile([128, 512], F32, tag="po")
                    nc.tensor.matmul(po[:], csu[:], r_u[:, hs],
                                     start=True, stop=False)
                    nc.tensor.matmul(po[:], csv[:], r_v[:, hs],
                                     start=False, stop=False)
                    nc.tensor.matmul(po[:], cs1[:], dbsb[:, hs],
                                     start=False, stop=True)
                    nc.vector.tensor_copy(outsb[:, hs], po[:])
                dma.dma_start(out[:], outsb[:])

    return nc


def kernel(**inputs) -> np.ndarray:
    _install_birfix()
    from concourse.bass_utils import run_bass_kernel_spmd
    import ml_dtypes

    bf16 = ml_dtypes.bfloat16

    X = np.asarray(inputs["observed_trajectory"], dtype=np.float32)
    W1 = np.asarray(inputs["W1"], dtype=np.float32)
    b1 = np.asarray(inputs["b1"], dtype=np.float32)
    W2 = np.asarray(inputs["W2"], dtype=np.float32)
    b2 = np.asarray(inputs["b2"], dtype=np.float64)
    freqs = np.asarray(inputs["freqs"], dtype=np.float64)
    dec_W = np.asarray(inputs["dec_W"], dtype=np.float32)
    dec_b = np.asarray(inputs["dec_b"], dtype=np.float32)
    num_steps = int(np.asarray(inputs["num_steps"]))
    T, D_ = X.shape
    assert (T, D_, num_steps) == (T_FULL, D, S_OUT), (T, D_, num_steps)

    base = freqs * TWO_PI * DT            # (A,) rad/step
    baseT = base / TWO_PI                 # turns/step

    t0 = T - W
    Xw = X[t0:]                            # (W, D)
    xT = np.ascontiguousarray(Xw.T)        # (D, W) fp32
    xhi = xT.astype(bf16)
    xlo = (xT - xhi.astype(np.float32)).astype(bf16)
    w1T = np.ascontiguousarray(W1.T)       # (D, D)
    w1hi = w1T.astype(bf16)
    w1lo = (w1T - w1hi.astype(np.float32)).astype(bf16)
    w2T = np.ascontiguousarray(W2.T)       # (D, A)
    w2hi = w2T.astype(bf16)
    w2lo = (w2T - w2hi.astype(np.float32)).astype(bf16)
    b1g = np.ascontiguousarray(b1.reshape(8, 128).T)

    j = np.arange(W, dtype=np.float64)
    ob = b2[:, None] / TWO_PI - j[None, :] * baseT[:, None]
    obT = (ob - np.round(ob)).astype(np.float32)        # (A, W) wrapped turns

    gv = np.arange(G, dtype=np.float32) - 64.0
    iotaZ0 = np.broadcast_to(gv / G, (128, G)).astype(np.float32).copy()
    iotaV = np.broadcast_to(gv, (128, G)).astype(np.float32).copy()
    iotaU = np.broadcast_to(np.arange(G + 1, dtype=np.float32) - 64.0,
                            (A, G + 1)).astype(np.float32).copy()

    decwt = np.ascontiguousarray(dec_W.T)
    decb = np.ascontiguousarray(dec_b.reshape(1, D))

    in_maps = []
    rows = S_OUT // NCORES
    for c in range(NCORES):
        s = np.arange(c * rows, (c + 1) * rows, dtype=np.float64)
        th = TWO_PI * ((W + s[None, :] + 1.0) * baseT[:, None])   # (A, rows)
        csm = np.empty((17, rows), np.float32)
        csm[0:8] = np.cos(th)
        csm[8:16] = -np.sin(th)
        csm[16] = 1.0
        in_maps.append({
            "xhiT": xhi, "xloT": np.ascontiguousarray(xlo[:, SPLIT:]),
            "w1hiT": w1hi, "w1loT": w1lo,
            "w2hiT": w2hi, "w2loT": w2lo,
            "b1g": b1g, "obT": obT,
            "iotaZ0": iotaZ0, "iotaV": iotaV, "iotaU": iotaU,
            "decwt": decwt, "decb": decb,
            "cs": np.ascontiguousarray(csm),
        })

    if "nc" not in _cache:
        _cache["nc"] = _build_nc()
    res = run_bass_kernel_spmd(_cache["nc"], in_maps, core_ids=list(range(NCORES)))
    out = np.concatenate([r["out"] for r in res.results], axis=0)
    return out.astype(np.float32)


# revision 19
# speedup vs baseline: 1.1530x; 1.0658x over previous
"""Trainium2 Bass kernel for nn_MirrorResonance.

Math summary
------------
reference: H = tanh(X @ W1.T + b1); E = H @ W2.T + b2; o = wrap(E)
           phases: p <- mod(p + base + K*sin(o_t - p), 2pi)  over T=16384 steps
           out[s] = cos(phases + (s+1)*base) @ dec_W.T + dec_b

Design (all phase math in fp32 TURN units, z-transformed z_j = p_j - j*base):
 * Truncation: the scan contracts (~0.971/step); only the last W=576 steps
   matter (validated: 2.4e-4 rel err of exact truncated scan).
 * Encoder: hybrid precision. X/W1 split into bf16 hi+lo; early window
   columns (noise contracted away) use 1 product (hi*hi), late columns use 3
   (hi*hi + lo*hi + hi*lo) which restores ~fp32 accuracy. Same for H@W2.
 * Parallel-in-time scan: split W into M=16 chunks of L=36 steps, one
   partition group (8 attractors) per chunk, and run each chunk's scan from
   G=128 grid initial conditions living in the free axis. All chunks advance
   in lockstep: L serial steps instead of W.
   Step (d-state = wrapped angle diff, turns): s=sin(2pi d);
   q = d - (K/2pi)s; d' = wrap(q + delta_j) via round-to-int32 trick.
 * Each chunk yields its end-map F_c on the grid. Maps are unwrapped along
   the grid axis with one hardware scan instruction (cumsum of wrapped
   neighbor diffs), converted to deviation-from-identity psi in grid units.
 * Stitch: 16 sequential tent-weight interpolations (|.|/relu/fused
   multiply-reduce) compose the chunk maps at the running boundary estimate.
 * Generation: rank-17 matmul with host-precomputed cos/sin(t*base) rows,
   sharded over the 8 cores (128 output rows each).

Full-pipeline numpy simulation of this exact algorithm: rel err 8.8e-4.
"""

import numpy as np

TWO_PI = 2.0 * np.pi
DT = 0.01
K = 0.5
NCORES = 8
T_FULL = 16384
D = 1024
A = 8
S_OUT = 1024

W = 576          # scan window
M = 16           # chunks
L = W // M       # serial steps (36)
G = 128          # grid points per chunk map
SPLIT = 396      # window column where high-precision encoder starts
CH = 396         # encoder column chunk (psum-bank sized)

_cache = {}


# ---------------------------------------------------------------------------
# BIR legalization: this walrus build supports at most ONE sync-wait per
# instruction; split extra waits into single-wait EventSemaphore predecessors
# on the same engine (semantics preserved: engine stalls before the op).
# ---------------------------------------------------------------------------
def _install_birfix():
    if _cache.get("birfix"):
        return
    import orjson
    import concourse.bass_utils as bu
    import concourse.bass2jax as b2j

    orig = bu.compile_bir_kernel

    def _legalize(bir: bytes) -> bytes:
        d = orjson.loads(bir)
        for fn in d.get("functions", []):
            for blk in fn.get("blocks", []):
                out = []
                for inst in blk.get("instructions", []):
                    si = inst.get("sync_info") or {}
                    waits = si.get("on_wait") or []
                    if len(waits) > 1:
                        for k, w in enumerate(waits[:-1]):
                            out.append({
                                "debug": inst.get("debug", 0),
                                "engine": inst["engine"],
                                "ins": [], "outs": [],
                                "name": f"{inst['name']}_w{k}",
                                "opcode": "EventSemaphore",
                                "sync_info": {"on_update": [], "on_wait": [w]},
                            })
                        si["on_wait"] = [waits[-1]]
                    out.append(inst)
                blk["instructions"] = out
        return orjson.dumps(d)

    def wrapped(bir_json: bytes, tmpdir: str, neff_name="file.neff"):
        return orig(_legalize(bir_json), tmpdir, neff_name)

    bu.compile_bir_kernel = wrapped
    b2j.compile_bir_kernel = wrapped
    _cache["birfix"] = True


def _build_nc():
    import concourse.bass as bass
    import concourse.tile as tile
    import concourse.mybir as mybir
    from concourse.alu_op_type import AluOpType as OP

    F32 = mybir.dt.float32
    I32 = mybir.dt.int32
    BF16 = mybir.dt.bfloat16
    AF = mybir.ActivationFunctionType
    INV_2PI = float(1.0 / TWO_PI)
    KT = float(K / TWO_PI)
    HALF_PI = float(np.pi / 2.0)
    GF = float(G)
    LATE = W - SPLIT  # columns in the high-precision range

    nc = bass.Bass("TRN2")
    xhiT = nc.dram_tensor("xhiT", [D, W], BF16, kind="ExternalInput")
    xloT = nc.dram_tensor("xloT", [D, LATE], BF16, kind="ExternalInput")
    w1hiT = nc.dram_tensor("w1hiT", [D, D], BF16, kind="ExternalInput")
    w1loT = nc.dram_tensor("w1loT", [D, D], BF16, kind="ExternalInput")
    w2hiT = nc.dram_tensor("w2hiT", [D, A], BF16, kind="ExternalInput")
    w2loT = nc.dram_tensor("w2loT", [D, A], BF16, kind="ExternalInput")
    b1g = nc.dram_tensor("b1g", [128, 8], F32, kind="ExternalInput")
    obT_d = nc.dram_tensor("obT", [A, W], F32, kind="ExternalInput")
    iotaZ0_d = nc.dram_tensor("iotaZ0", [128, G], F32, kind="ExternalInput")
    iotaV_d = nc.dram_tensor("iotaV", [128, G], F32, kind="ExternalInput")
    iotaU_d = nc.dram_tensor("iotaU", [A, G + 1], F32, kind="ExternalInput")
    decwt = nc.dram_tensor("decwt", [A, D], F32, kind="ExternalInput")
    decb = nc.dram_tensor("decb", [1, D], F32, kind="ExternalInput")
    cs = nc.dram_tensor("cs", [17, 128], F32, kind="ExternalInput")
    out = nc.dram_tensor("out", [128, D], F32, kind="ExternalOutput")

    with tile.TileContext(nc) as tc:
        with (
            tc.tile_pool(name="sb", bufs=1) as sb,
            tc.tile_pool(name="ps", bufs=2, space="PSUM") as ps,
            tc.tile_pool(name="ps1", bufs=1, space="PSUM") as ps1,
        ):
            PC = SPLIT // L          # first high-precision chunk (11)
            NP = M - PC              # number of precise chunks (5)
            w1h = [sb.tile([128, D], BF16, name=f"w1h{k}", tag=f"w1h{k}") for k in range(8)]
            w1l = [sb.tile([128, D], BF16, name=f"w1l{k}", tag=f"w1l{k}") for k in range(8)]
            xh = [sb.tile([128, M, L], BF16, name=f"xh{k}", tag=f"xh{k}") for k in range(8)]
            xl = [sb.tile([128, NP, L], BF16, name=f"xl{k}", tag=f"xl{k}") for k in range(8)]
            w2h = sb.tile([128, 8, A], BF16)
            w2l = sb.tile([128, 8, A], BF16)
            b1sb = sb.tile([128, 8], F32)
            obT = sb.tile([A, M, L], F32)
            hf = sb.tile([128, NP, L // 2], F32)
            hhi = [sb.tile([128, M, L], BF16, name=f"hhi{n}", tag=f"hhi{n}") for n in range(8)]
            hlo = [sb.tile([128, NP, L], BF16, name=f"hlo{n}", tag=f"hlo{n}") for n in range(8)]
            e23 = sb.tile([A, M, L], F32)
            rE3 = sb.tile([A, M, L], I32)
            oTt3 = sb.tile([A, M, L], F32)
            oR = sb.tile([128, L], F32)
            dd = sb.tile([128, L - 1], F32)
            rdd = sb.tile([128, L - 1], I32)
            dR = sb.tile([128, L - 1], F32)
            iotaZ0 = sb.tile([128, G], F32)
            iotaV = sb.tile([128, G], F32)
            iotaU = sb.tile([A, G + 1], F32)
            dst = sb.tile([128, G], F32)
            s_t = sb.tile([128, G], F32)
            q_t = sb.tile([128, G], F32)
            qq_t = sb.tile([128, G], F32)
            r_t = sb.tile([128, G], I32)
            zL = sb.tile([128, G], F32)
            wdt = sb.tile([128, G - 1], F32)
            rwd = sb.tile([128, G - 1], I32)
            ones_t = sb.tile([128, G - 1], F32)
            zu = sb.tile([128, G], F32)
            psig = sb.tile([128, G + 1], F32)
            w1e = sb.tile([128, 1], F32)
            rex = sb.tile([128, 1], I32)
            w2e = sb.tile([128, 1], F32)
            zue = sb.tile([128, 1], F32)
            zmapS = sb.tile([A, M * (G + 1)], F32)
            xst = [sb.tile([A, 1], F32, name=f"xst{i}") for i in range(2)]
            rg_t = sb.tile([A, 1], I32)
            xf_t = sb.tile([A, 1], F32)
            dab_t = sb.tile([A, G + 1], F32)
            wt_t = sb.tile([A, G + 1], F32)
            scr_t = sb.tile([A, G + 1], F32)
            y_t = sb.tile([A, 1], F32)
            one8 = sb.tile([A, 1], F32)
            cpih = sb.tile([A, 1], F32)
            zt_t = sb.tile([A, 1], F32)
            rT_t = sb.tile([A, 1], I32)
            zw_t = sb.tile([A, 1], F32)
            az_t = sb.tile([A, 1], F32)
            uc_t = sb.tile([A, 1], F32)
            vs_t = sb.tile([A, 1], F32)
            dwsb = sb.tile([A, D], F32)
            r_u = sb.tile([A, D], F32)
            r_v = sb.tile([A, D], F32)
            dbsb = sb.tile([1, D], F32)
            csu = sb.tile([A, 128], F32)
            csv = sb.tile([A, 128], F32)
            cs1 = sb.tile([1, 128], F32)
            outsb = sb.tile([128, D], F32)

            dma = nc.sync
            for k in range(8):
                dma.dma_start(xh[k][:], xhiT[k * 128:(k + 1) * 128, :])
                dma.dma_start(w1h[k][:], w1hiT[k * 128:(k + 1) * 128, :])
            for k in range(8):
                dma.dma_start(xl[k][:], xloT[k * 128:(k + 1) * 128, :])
                dma.dma_start(w1l[k][:], w1loT[k * 128:(k + 1) * 128, :])
                dma.dma_start(w2h[:, k, :], w2hiT[k * 128:(k + 1) * 128, :])
                dma.dma_start(w2l[:, k, :], w2loT[k * 128:(k + 1) * 128, :])
            dma.dma_start(b1sb[:], b1g[:])
            dma.dma_start(obT[:], obT_d[:])
            HB = L // 2   # scan steps per encoder block (18)
            dma.dma_start(iotaZ0[:], iotaZ0_d[:])
            dma.dma_start(iotaV[:], iotaV_d[:])
            dma.dma_start(iotaU[:], iotaU_d[:])
            dma.dma_start(dwsb[:], decwt[:])
            dma.dma_start(dbsb[:], decb[:])
            dma.dma_start(csu[:], cs[0:8, :])
            dma.dma_start(csv[:], cs[8:16, :])
            dma.dma_start(cs1[:], cs[16:17, :])
            nc.vector.memset(ones_t[:], 1.0)
            nc.vector.memset(one8[:], 1.0)
            nc.vector.memset(cpih[:], HALF_PI)
            nc.vector.memset(xst[0][:], 0.0)

            # ------- encoder, in two scan-step blocks so the scan overlaps -------
            with nc.named_scope("encoder"):
                qs = (nc.sync, nc.scalar, nc.gpsimd)
                for b in range(2):
                    isl = slice(b * HB, (b + 1) * HB)
                    pec = ps1.tile([A, PC, HB], F32, tag="pec")
                    pep = ps1.tile([A, NP, HB], F32, tag="pep")
                    for nt in range(8):
                        nsl = slice(nt * 128, (nt + 1) * 128)
                        phc = ps.tile([128, PC, HB], F32, tag="phc")
                        php = ps.tile([128, NP, HB], F32, tag="php")
                        for kt in range(8):
                            last = kt == 7
                            nc.tensor.matmul(phc[:], w1h[kt][:, nsl],
                                             xh[kt][:, 0:PC, isl],
                                             start=(kt == 0), stop=last)
                            nc.tensor.matmul(php[:], w1h[kt][:, nsl],
                                             xh[kt][:, PC:M, isl],
                                             start=(kt == 0), stop=False)
                            nc.tensor.matmul(php[:], w1h[kt][:, nsl],
                                             xl[kt][:, :, isl],
                                             start=False, stop=False)
                            nc.tensor.matmul(php[:], w1l[kt][:, nsl],
                                             xh[kt][:, PC:M, isl],
                                             start=False, stop=last)
                        nc.scalar.activation(hhi[nt][:, 0:PC, isl], phc[:],
                                             AF.Tanh,
                                             bias=b1sb[:, nt:nt + 1], scale=1.0)
                        nc.scalar.activation(hf[:], php[:], AF.Tanh,
                                             bias=b1sb[:, nt:nt + 1], scale=1.0)
                        nc.vector.tensor_scalar(hhi[nt][:, PC:M, isl], hf[:],
                                                1.0, 0.0, OP.mult, OP.add)
                        nc.vector.scalar_tensor_tensor(hlo[nt][:, :, isl],
                                                       hhi[nt][:, PC:M, isl],
                                                       -1.0, hf[:],
                                                       OP.mult, OP.add)
                    for nt in range(8):
                        nc.tensor.matmul(pec[:], w2h[:, nt, :],
                                         hhi[nt][:, 0:PC, isl],
                                         start=(nt == 0), stop=(nt == 7))
                        nc.tensor.matmul(pep[:], w2h[:, nt, :],
                                         hhi[nt][:, PC:M, isl],
                                         start=(nt == 0), stop=False)
                        nc.tensor.matmul(pep[:], w2h[:, nt, :],
                                         hlo[nt][:, :, isl],
                                         start=False, stop=False)
                        nc.tensor.matmul(pep[:], w2l[:, nt, :],
                                         hhi[nt][:, PC:M, isl],
                                         start=False, stop=(nt == 7))
                    # e2 = E/2pi + obT (turns), wrap, rearrange, deltas
                    nc.vector.scalar_tensor_tensor(e23[:, 0:PC, isl], pec[:],
                                                   INV_2PI, obT[:, 0:PC, isl],
                                                   OP.mult, OP.add)
                    nc.vector.scalar_tensor_tensor(e23[:, PC:M, isl], pep[:],
                                                   INV_2PI, obT[:, PC:M, isl],
                                                   OP.mult, OP.add)
                    nc.vector.tensor_scalar(rE3[:, :, isl], e23[:, :, isl],
                                            1.0, 0.0, OP.mult, OP.add)
                    nc.vector.scalar_tensor_tensor(oTt3[:, :, isl], rE3[:, :, isl],
                                                   -1.0, e23[:, :, isl],
                                                   OP.mult, OP.add)
                    for c in range(M):
                        qs[c % 3].dma_start(oR[c * 8:(c + 1) * 8, isl],
                                            oTt3[:, c, isl])
                    dsl = slice(b * HB - (1 if b else 0), (b + 1) * HB - 1)
                    nc.vector.tensor_tensor(dd[:, dsl], oR[:, dsl.start + 1:dsl.stop + 1],
                                            oR[:, dsl.start:dsl.stop], OP.subtract)
                    nc.vector.tensor_scalar(rdd[:, dsl], dd[:, dsl], 1.0, 0.0,
                                            OP.mult, OP.add)
                    nc.vector.scalar_tensor_tensor(dR[:, dsl], rdd[:, dsl], -1.0,
                                                   dd[:, dsl], OP.mult, OP.add)

            # ---------------- grid scan ----------------
            with nc.named_scope("scan"):
                # d0 = wrap(oR[:,0] - z0grid)
                nc.vector.tensor_scalar(q_t[:], iotaZ0[:], -1.0, oR[:, 0:1],
                                        OP.mult, OP.add)
                nc.vector.tensor_scalar(r_t[:], q_t[:], 1.0, 0.0, OP.mult, OP.add)
                nc.vector.scalar_tensor_tensor(dst[:], r_t[:], -1.0, q_t[:],
                                               OP.mult, OP.add)
                for j in range(L - 1):
                    nc.scalar.activation(s_t[:], dst[:], AF.Sin,
                                         bias=0.0, scale=TWO_PI)
                    nc.vector.scalar_tensor_tensor(q_t[:], s_t[:], -KT, dst[:],
                                                   OP.mult, OP.add)
                    # qq and its rounding both depend only on q: issue
                    # back-to-back so they pipeline on the DVE
                    nc.vector.tensor_scalar(qq_t[:], q_t[:], dR[:, j:j + 1], 0.0,
                                            OP.add, OP.add)
                    nc.vector.tensor_scalar(r_t[:], q_t[:], dR[:, j:j + 1], 0.0,
                                            OP.add, OP.add)
                    nc.vector.scalar_tensor_tensor(dst[:], r_t[:], -1.0, qq_t[:],
                                                   OP.mult, OP.add)
                # final partial step -> map values zL
                nc.scalar.activation(s_t[:], dst[:], AF.Sin, bias=0.0, scale=TWO_PI)
                nc.vector.scalar_tensor_tensor(q_t[:], s_t[:], -KT, dst[:],
                                               OP.mult, OP.add)
                nc.vector.tensor_scalar(zL[:], q_t[:], -1.0, oR[:, L - 1:L],
                                        OP.mult, OP.add)

            # ---------------- unwrap maps + psi ----------------
            with nc.named_scope("maps"):
                nc.vector.tensor_tensor(wdt[:], zL[:, 1:G], zL[:, 0:G - 1],
                                        OP.subtract)
                nc.vector.tensor_scalar(rwd[:], wdt[:], 1.0, 0.0, OP.mult, OP.add)
                nc.vector.scalar_tensor_tensor(wdt[:], rwd[:], -1.0, wdt[:],
                                               OP.mult, OP.add)
                nc.vector.tensor_copy(zu[:, 0:1], zL[:, 0:1])
                nc.vector.tensor_tensor_scan(zu[:, 1:G], ones_t[:], wdt[:],
                                             zL[:, 0:1], OP.mult, OP.add)
                nc.vector.scalar_tensor_tensor(psig[:, 0:G], zu[:], GF, iotaV[:],
                                               OP.mult, OP.subtract)
                # extension column (grid point +64)
                nc.vector.tensor_tensor(w1e[:], zL[:, 0:1], zL[:, G - 1:G],
                                        OP.subtract)
                nc.vector.tensor_scalar(rex[:], w1e[:], 1.0, 0.0, OP.mult, OP.add)
                nc.vector.scalar_tensor_tensor(w2e[:], rex[:], -1.0, w1e[:],
                                               OP.mult, OP.add)
                nc.vector.tensor_tensor(zue[:], zu[:, G - 1:G], w2e[:], OP.add)
                nc.vector.tensor_scalar(psig[:, G:G + 1], zue[:], GF, 64.0,
                                        OP.mult, OP.subtract)
                qs = (nc.sync, nc.scalar, nc.gpsimd)
                for c in range(M):
                    qs[c % 3].dma_start(zmapS[:, c * (G + 1):(c + 1) * (G + 1)],
                                        psig[c * 8:(c + 1) * 8, :])

            # ---------------- stitch ----------------
            with nc.named_scope("stitch"):
                for c in range(M):
                    xin = xst[c % 2][:]
                    xout = xst[(c + 1) % 2][:]
                    nc.vector.tensor_scalar(rg_t[:], xin, float(1.0 / G), 0.0,
                                            OP.mult, OP.add)
                    nc.vector.scalar_tensor_tensor(xf_t[:], rg_t[:], -GF, xin,
                                                   OP.mult, OP.add)
                    nc.scalar.activation(dab_t[:], iotaU[:], AF.Abs,
                                         bias=xf_t[:], scale=-1.0)
                    nc.scalar.activation(wt_t[:], dab_t[:], AF.Relu,
                                         bias=one8[:], scale=-1.0)
                    nc.vector.scalar_tensor_tensor(
                        scr_t[:], wt_t[:], 1.0,
                        zmapS[:, c * (G + 1):(c + 1) * (G + 1)],
                        OP.mult, OP.mult, accum_out=y_t[:])
                    nc.vector.tensor_tensor(xout, y_t[:], xin, OP.add)

            # ---------------- tail: generation ----------------
            with nc.named_scope("tail"):
                xfin = xst[M % 2][:]
                nc.vector.tensor_scalar(zt_t[:], xfin, float(1.0 / G), 0.0,
                                        OP.mult, OP.add)
                nc.vector.tensor_scalar(rT_t[:], zt_t[:], 1.0, 0.0, OP.mult, OP.add)
                nc.vector.scalar_tensor_tensor(zw_t[:], rT_t[:], -1.0, zt_t[:],
                                               OP.mult, OP.add)
                nc.scalar.activation(az_t[:], zw_t[:], AF.Abs, bias=0.0, scale=1.0)
                nc.scalar.activation(uc_t[:], az_t[:], AF.Sin,
                                     bias=cpih[:], scale=-TWO_PI)   # cos(2pi z)
                nc.scalar.activation(vs_t[:], zw_t[:], AF.Sin,
                                     bias=0.0, scale=TWO_PI)        # sin(2pi z)
                nc.scalar.activation(r_u[:], dwsb[:], AF.Copy,
                                     bias=0.0, scale=uc_t[:])
                nc.vector.tensor_scalar(r_v[:], dwsb[:], vs_t[:], 0.0,
                                        OP.mult, OP.add)
                for half in range(2):
                    hs = slice(half * 512, (half + 1) * 512)
                    po = ps.tile([128, 512], F32, tag="po")
                    nc.tensor.matmul(po[:], csu[:], r_u[:, hs],
                                     start=True, stop=False)
                    nc.tensor.matmul(po[:], csv[:], r_v[:, hs],
                                     start=False, stop=False)
                    nc.tensor.matmul(po[:], cs1[:], dbsb[:, hs],
                                     start=False, stop=True)
                    nc.vector.tensor_copy(outsb[:, hs], po[:])
                    dma.dma_start(out[0:128, hs], outsb[:, hs])

    return nc


def kernel(**inputs) -> np.ndarray:
    _install_birfix()
    from concourse.bass_utils import run_bass_kernel_spmd
    import ml_dtypes

    bf16 = ml_dtypes.bfloat16

    X = np.asarray(inputs["observed_trajectory"], dtype=np.float32)
    W1 = np.asarray(inputs["W1"], dtype=np.float32)
    b1 = np.asarray(inputs["b1"], dtype=np.float32)
    W2 = np.asarray(inputs["W2"], dtype=np.float32)
    b2 = np.asarray(inputs["b2"], dtype=np.float64)
    freqs = np.asarray(inputs["freqs"], dtype=np.float64)
    dec_W = np.asarray(inputs["dec_W"], dtype=np.float32)
    dec_b = np.asarray(inputs["dec_b"], dtype=np.float32)
    num_steps = int(np.asarray(inputs["num_steps"]))
    T, D_ = X.shape
    assert (T, D_, num_steps) == (T_FULL, D, S_OUT), (T, D_, num_steps)

    base = freqs * TWO_PI * DT            # (A,) rad/step
    baseT = base / TWO_PI                 # turns/step

    t0 = T - W
    Xw = X[t0:]                            # (W, D)
    xT = np.ascontiguousarray(Xw.T)        # (D, W) fp32
    xhi = xT.astype(bf16)
    xlo = (xT - xhi.astype(np.float32)).astype(bf16)
    w1T = np.ascontiguousarray(W1.T)       # (D, D)
    w1hi = w1T.astype(bf16)
    w1lo = (w1T - w1hi.astype(np.float32)).astype(bf16)
    w2T = np.ascontiguousarray(W2.T)       # (D, A)
    w2hi = w2T.astype(bf16)
    w2lo = (w2T - w2hi.astype(np.float32)).astype(bf16)
    b1g = np.ascontiguousarray(b1.reshape(8, 128).T)

    j = np.arange(W, dtype=np.float64)
    ob = b2[:, None] / TWO_PI - j[None, :] * baseT[:, None]
    obT = (ob - np.round(ob)).astype(np.float32)        # (A, W) wrapped turns

    gv = np.arange(G, dtype=np.float32) - 64.0
    iotaZ0 = np.broadcast_to(gv / G, (128, G)).astype(np.float32).copy()
    iotaV = np.broadcast_to(gv, (128, G)).astype(np.float32).copy()
    iotaU = np.broadcast_to(np.arange(G + 1, dtype=np.float32) - 64.0,
                            (A, G + 1)).astype(np.float32).copy()

    decwt = np.ascontiguousarray(dec_W.T)
    decb = np.ascontiguousarray(dec_b.reshape(1, D))

    in_maps = []
    rows = S_OUT // NCORES
    for c in range(NCORES):
        s = np.arange(c * rows, (c + 1) * rows, dtype=np.float64)
        th = TWO_PI * ((W + s[None, :] + 1.0) * baseT[:, None])   # (A, rows)
        csm = np.empty((17, rows), np.float32)
        csm[0:8] = np.cos(th)
        csm[8:16] = -np.sin(th)
        csm[16] = 1.0
        in_maps.append({
            "xhiT": xhi, "xloT": np.ascontiguousarray(xlo[:, SPLIT:]),
            "w1hiT": w1hi, "w1loT": w1lo,
            "w2hiT": w2hi, "w2loT": w2lo,
            "b1g": b1g, "obT": obT,
            "iotaZ0": iotaZ0, "iotaV": iotaV, "iotaU": iotaU,
            "decwt": decwt, "decb": decb,
            "cs": np.ascontiguousarray(csm),
        })

    if "nc" not in _cache:
        _cache["nc"] = _build_nc()
    res = run_bass_kernel_spmd(_cache["nc"], in_maps, core_ids=list(range(NCORES)))
    out = np.concatenate([r["out"] for r in res.results], axis=0)
    return out.astype(np.float32)


# revision 20
# speedup vs baseline: 1.2307x; 1.0673x over previous
"""Trainium2 Bass kernel for nn_MirrorResonance.

Math summary
------------
reference: H = tanh(X @ W1.T + b1); E = H @ W2.T + b2; o = wrap(E)
           phases: p <- mod(p + base + K*sin(o_t - p), 2pi)  over T=16384 steps
           out[s] = cos(phases + (s+1)*base) @ dec_W.T + dec_b

Design (all phase math in fp32 TURN units, z-transformed z_j = p_j - j*base):
 * Truncation: the scan contracts (~0.971/step); only the last W=576 steps
   matter (validated: 2.4e-4 rel err of exact truncated scan).
 * Encoder: hybrid precision. X/W1 split into bf16 hi+lo; early window
   columns (noise contracted away) use 1 product (hi*hi), late columns use 3
   (hi*hi + lo*hi + hi*lo) which restores ~fp32 accuracy. Same for H@W2.
 * Parallel-in-time scan: split W into M=16 chunks of L=36 steps, one
   partition group (8 attractors) per chunk, and run each chunk's scan from
   G=128 grid initial conditions living in the free axis. All chunks advance
   in lockstep: L serial steps instead of W.
   Step (d-state = wrapped angle diff, turns): s=sin(2pi d);
   q = d - (K/2pi)s; d' = wrap(q + delta_j) via round-to-int32 trick.
 * Each chunk yields its end-map F_c on the grid. Maps are unwrapped along
   the grid axis with one hardware scan instruction (cumsum of wrapped
   neighbor diffs), converted to deviation-from-identity psi in grid units.
 * Stitch: 16 sequential tent-weight interpolations (|.|/relu/fused
   multiply-reduce) compose the chunk maps at the running boundary estimate.
 * Generation: rank-17 matmul with host-precomputed cos/sin(t*base) rows,
   sharded over the 8 cores (128 output rows each).

Full-pipeline numpy simulation of this exact algorithm: rel err 8.8e-4.
"""

import numpy as np

TWO_PI = 2.0 * np.pi
DT = 0.01
K = 0.5
NCORES = 8
T_FULL = 16384
D = 1024
A = 8
S_OUT = 1024

W = 576          # scan window
M = 16           # chunks
L = W // M       # serial steps (36)
G = 128          # grid points per chunk map
SPLIT = 396      # window column where high-precision encoder starts
CH = 396         # encoder column chunk (psum-bank sized)

_cache = {}


# ---------------------------------------------------------------------------
# BIR legalization: this walrus build supports at most ONE sync-wait per
# instruction; split extra waits into single-wait EventSemaphore predecessors
# on the same engine (semantics preserved: engine stalls before the op).
# ---------------------------------------------------------------------------
def _install_birfix():
    if _cache.get("birfix"):
        return
    import orjson
    import concourse.bass_utils as bu
    import concourse.bass2jax as b2j

    orig = bu.compile_bir_kernel

    def _legalize(bir: bytes) -> bytes:
        d = orjson.loads(bir)
        for fn in d.get("functions", []):
            for blk in fn.get("blocks", []):
                out = []
                for inst in blk.get("instructions", []):
                    si = inst.get("sync_info") or {}
                    waits = si.get("on_wait") or []
                    if len(waits) > 1:
                        for k, w in enumerate(waits[:-1]):
                            out.append({
                                "debug": inst.get("debug", 0),
                                "engine": inst["engine"],
                                "ins": [], "outs": [],
                                "name": f"{inst['name']}_w{k}",
                                "opcode": "EventSemaphore",
                                "sync_info": {"on_update": [], "on_wait": [w]},
                            })
                        si["on_wait"] = [waits[-1]]
                    out.append(inst)
                blk["instructions"] = out
        return orjson.dumps(d)

    def wrapped(bir_json: bytes, tmpdir: str, neff_name="file.neff"):
        return orig(_legalize(bir_json), tmpdir, neff_name)

    bu.compile_bir_kernel = wrapped
    b2j.compile_bir_kernel = wrapped
    _cache["birfix"] = True


def _build_nc():
    import concourse.bass as bass
    import concourse.tile as tile
    import concourse.mybir as mybir
    from concourse.alu_op_type import AluOpType as OP

    F32 = mybir.dt.float32
    I32 = mybir.dt.int32
    BF16 = mybir.dt.bfloat16
    AF = mybir.ActivationFunctionType
    INV_2PI = float(1.0 / TWO_PI)
    KT = float(K / TWO_PI)
    SC18 = float(2 ** 18)
    C18 = float(TWO_PI / 2 ** 18)
    K18 = float(K / TWO_PI * 2 ** 18)
    MASK18 = (1 << 18) - 1
    NEG_PI = float(-np.pi)
    HALF_PI = float(np.pi / 2.0)
    GF = float(G)
    LATE = W - SPLIT  # columns in the high-precision range

    nc = bass.Bass("TRN2")
    xhiT = nc.dram_tensor("xhiT", [D, W], BF16, kind="ExternalInput")
    xloT = nc.dram_tensor("xloT", [D, LATE], BF16, kind="ExternalInput")
    w1hiT = nc.dram_tensor("w1hiT", [D, D], BF16, kind="ExternalInput")
    w1loT = nc.dram_tensor("w1loT", [D, D], BF16, kind="ExternalInput")
    w2hiT = nc.dram_tensor("w2hiT", [D, A], BF16, kind="ExternalInput")
    w2loT = nc.dram_tensor("w2loT", [D, A], BF16, kind="ExternalInput")
    b1g = nc.dram_tensor("b1g", [128, 8], F32, kind="ExternalInput")
    obT_d = nc.dram_tensor("obT", [A, W], F32, kind="ExternalInput")
    iotaZ0_d = nc.dram_tensor("iotaZ0", [128, G], F32, kind="ExternalInput")
    iotaV_d = nc.dram_tensor("iotaV", [128, G], F32, kind="ExternalInput")
    iotaU_d = nc.dram_tensor("iotaU", [A, G + 1], F32, kind="ExternalInput")
    decwt = nc.dram_tensor("decwt", [A, D], F32, kind="ExternalInput")
    decb = nc.dram_tensor("decb", [1, D], F32, kind="ExternalInput")
    cs = nc.dram_tensor("cs", [17, 128], F32, kind="ExternalInput")
    out = nc.dram_tensor("out", [128, D], F32, kind="ExternalOutput")

    with tile.TileContext(nc) as tc:
        with (
            tc.tile_pool(name="sb", bufs=1) as sb,
            tc.tile_pool(name="ps", bufs=2, space="PSUM") as ps,
            tc.tile_pool(name="ps1", bufs=1, space="PSUM") as ps1,
        ):
            PC = SPLIT // L          # first high-precision chunk (11)
            NP = M - PC              # number of precise chunks (5)
            w1h = [sb.tile([128, D], BF16, name=f"w1h{k}", tag=f"w1h{k}") for k in range(8)]
            w1l = [sb.tile([128, D], BF16, name=f"w1l{k}", tag=f"w1l{k}") for k in range(8)]
            xh = [sb.tile([128, M, L], BF16, name=f"xh{k}", tag=f"xh{k}") for k in range(8)]
            xl = [sb.tile([128, NP, L], BF16, name=f"xl{k}", tag=f"xl{k}") for k in range(8)]
            w2h = sb.tile([128, 8, A], BF16)
            w2l = sb.tile([128, 8, A], BF16)
            b1sb = sb.tile([128, 8], F32)
            obT = sb.tile([A, M, L], F32)
            hf = sb.tile([128, NP, L // 2], F32)
            hhi = [sb.tile([128, M, L], BF16, name=f"hhi{n}", tag=f"hhi{n}") for n in range(8)]
            hlo = [sb.tile([128, NP, L], BF16, name=f"hlo{n}", tag=f"hlo{n}") for n in range(8)]
            e23 = sb.tile([A, M, L], F32)
            rE3 = sb.tile([A, M, L], I32)
            oTt3 = sb.tile([A, M, L], F32)
            oR = sb.tile([128, L], F32)
            oR18 = sb.tile([128, L], F32)
            dd18 = sb.tile([128, L - 1], F32)
            mpi_t = sb.tile([128, 1], F32)
            iotaZ0 = sb.tile([128, G], F32)
            iotaV = sb.tile([128, G], F32)
            iotaU = sb.tile([A, G + 1], F32)
            dst = sb.tile([128, G], I32)
            s_t = sb.tile([128, G], F32)
            q_t = sb.tile([128, G], F32)
            r_t = sb.tile([128, G], I32)
            zL = sb.tile([128, G], F32)
            wdt = sb.tile([128, G - 1], F32)
            rwd = sb.tile([128, G - 1], I32)
            ones_t = sb.tile([128, G - 1], F32)
            zu = sb.tile([128, G], F32)
            psig = sb.tile([128, G + 1], F32)
            w1e = sb.tile([128, 1], F32)
            rex = sb.tile([128, 1], I32)
            w2e = sb.tile([128, 1], F32)
            zue = sb.tile([128, 1], F32)
            zmapS = sb.tile([A, M * (G + 1)], F32)
            xst = [sb.tile([A, 1], F32, name=f"xst{i}") for i in range(2)]
            rg_t = sb.tile([A, 1], I32)
            xf_t = sb.tile([A, 1], F32)
            dab_t = sb.tile([A, G + 1], F32)
            wt_t = sb.tile([A, G + 1], F32)
            scr_t = sb.tile([A, G + 1], F32)
            y_t = sb.tile([A, 1], F32)
            one8 = sb.tile([A, 1], F32)
            cpih = sb.tile([A, 1], F32)
            zt_t = sb.tile([A, 1], F32)
            rT_t = sb.tile([A, 1], I32)
            zw_t = sb.tile([A, 1], F32)
            az_t = sb.tile([A, 1], F32)
            uc_t = sb.tile([A, 1], F32)
            vs_t = sb.tile([A, 1], F32)
            dwsb = sb.tile([A, D], F32)
            r_u = sb.tile([A, D], F32)
            r_v = sb.tile([A, D], F32)
            dbsb = sb.tile([1, D], F32)
            csu = sb.tile([A, 128], F32)
            csv = sb.tile([A, 128], F32)
            cs1 = sb.tile([1, 128], F32)
            outsb = sb.tile([128, D], F32)

            dma = nc.sync
            for k in range(8):
                dma.dma_start(xh[k][:], xhiT[k * 128:(k + 1) * 128, :])
                dma.dma_start(w1h[k][:], w1hiT[k * 128:(k + 1) * 128, :])
            for k in range(8):
                dma.dma_start(xl[k][:], xloT[k * 128:(k + 1) * 128, :])
                dma.dma_start(w1l[k][:], w1loT[k * 128:(k + 1) * 128, :])
                dma.dma_start(w2h[:, k, :], w2hiT[k * 128:(k + 1) * 128, :])
                dma.dma_start(w2l[:, k, :], w2loT[k * 128:(k + 1) * 128, :])
            dma.dma_start(b1sb[:], b1g[:])
            dma.dma_start(obT[:], obT_d[:])
            HB = L // 2   # scan steps per encoder block (18)
            dma.dma_start(iotaZ0[:], iotaZ0_d[:])
            dma.dma_start(iotaV[:], iotaV_d[:])
            dma.dma_start(iotaU[:], iotaU_d[:])
            dma.dma_start(dwsb[:], decwt[:])
            dma.dma_start(dbsb[:], decb[:])
            dma.dma_start(csu[:], cs[0:8, :])
            dma.dma_start(csv[:], cs[8:16, :])
            dma.dma_start(cs1[:], cs[16:17, :])
            nc.vector.memset(ones_t[:], 1.0)
            nc.vector.memset(one8[:], 1.0)
            nc.vector.memset(cpih[:], HALF_PI)
            nc.vector.memset(xst[0][:], 0.0)
            nc.vector.memset(mpi_t[:], NEG_PI)

            # ------- encoder, in two scan-step blocks so the scan overlaps -------
            with nc.named_scope("encoder"):
                qs = (nc.sync, nc.scalar, nc.gpsimd)
                for b in range(2):
                    isl = slice(b * HB, (b + 1) * HB)
                    pec = ps1.tile([A, PC, HB], F32, tag="pec")
                    pep = ps1.tile([A, NP, HB], F32, tag="pep")
                    for nt in range(8):
                        nsl = slice(nt * 128, (nt + 1) * 128)
                        phc = ps.tile([128, PC, HB], F32, tag="phc")
                        php = ps.tile([128, NP, HB], F32, tag="php")
                        for kt in range(8):
                            last = kt == 7
                            nc.tensor.matmul(phc[:], w1h[kt][:, nsl],
                                             xh[kt][:, 0:PC, isl],
                                             start=(kt == 0), stop=last)
                            nc.tensor.matmul(php[:], w1h[kt][:, nsl],
                                             xh[kt][:, PC:M, isl],
                                             start=(kt == 0), stop=False)
                            nc.tensor.matmul(php[:], w1h[kt][:, nsl],
                                             xl[kt][:, :, isl],
                                             start=False, stop=False)
                            nc.tensor.matmul(php[:], w1l[kt][:, nsl],
                                             xh[kt][:, PC:M, isl],
                                             start=False, stop=last)
                        nc.scalar.activation(hhi[nt][:, 0:PC, isl], phc[:],
                                             AF.Tanh,
                                             bias=b1sb[:, nt:nt + 1], scale=1.0)
                        nc.scalar.activation(hf[:], php[:], AF.Tanh,
                                             bias=b1sb[:, nt:nt + 1], scale=1.0)
                        nc.vector.tensor_scalar(hhi[nt][:, PC:M, isl], hf[:],
                                                1.0, 0.0, OP.mult, OP.add)
                        nc.vector.scalar_tensor_tensor(hlo[nt][:, :, isl],
                                                       hhi[nt][:, PC:M, isl],
                                                       -1.0, hf[:],
                                                       OP.mult, OP.add)
                    for nt in range(8):
                        nc.tensor.matmul(pec[:], w2h[:, nt, :],
                                         hhi[nt][:, 0:PC, isl],
                                         start=(nt == 0), stop=(nt == 7))
                        nc.tensor.matmul(pep[:], w2h[:, nt, :],
                                         hhi[nt][:, PC:M, isl],
                                         start=(nt == 0), stop=False)
                        nc.tensor.matmul(pep[:], w2h[:, nt, :],
                                         hlo[nt][:, :, isl],
                                         start=False, stop=False)
                        nc.tensor.matmul(pep[:], w2l[:, nt, :],
                                         hhi[nt][:, PC:M, isl],
                                         start=False, stop=(nt == 7))
                    # e2 = E/2pi + obT (turns), wrap, rearrange, deltas
                    nc.vector.scalar_tensor_tensor(e23[:, 0:PC, isl], pec[:],
                                                   INV_2PI, obT[:, 0:PC, isl],
                                                   OP.mult, OP.add)
                    nc.vector.scalar_tensor_tensor(e23[:, PC:M, isl], pep[:],
                                                   INV_2PI, obT[:, PC:M, isl],
                                                   OP.mult, OP.add)
                    nc.vector.tensor_scalar(rE3[:, :, isl], e23[:, :, isl],
                                            1.0, 0.0, OP.mult, OP.add)
                    nc.vector.scalar_tensor_tensor(oTt3[:, :, isl], rE3[:, :, isl],
                                                   -1.0, e23[:, :, isl],
                                                   OP.mult, OP.add)
                    for c in range(M):
                        qs[c % 3].dma_start(oR[c * 8:(c + 1) * 8, isl],
                                            oTt3[:, c, isl])
                    nc.vector.tensor_scalar(oR18[:, isl], oR[:, isl], SC18, 0.0,
                                            OP.mult, OP.add)
                    dsl = slice(b * HB - (1 if b else 0), (b + 1) * HB - 1)
                    nc.vector.tensor_tensor(dd18[:, dsl],
                                            oR18[:, dsl.start + 1:dsl.stop + 1],
                                            oR18[:, dsl.start:dsl.stop], OP.subtract)

            # ------- grid scan: int32 phase state, AND-mod range reduction ----
            # state dst = d mod 1 turn, in 2^-18-turn units.  Sin(C18*dst - pi)
            # = -sin(2pi d): the sign is absorbed into +K18 below.
            with nc.named_scope("scan"):
                nc.vector.tensor_scalar(q_t[:], iotaZ0[:], -SC18, oR18[:, 0:1],
                                        OP.mult, OP.add)
                nc.vector.tensor_scalar(r_t[:], q_t[:], 1.0, 0.0, OP.mult, OP.add)
                nc.vector.tensor_scalar(dst[:], r_t[:], MASK18, MASK18,
                                        OP.bitwise_and, OP.bitwise_and)
                for j in range(L - 1):
                    nc.scalar.activation(s_t[:], dst[:], AF.Sin,
                                         bias=mpi_t[:], scale=C18)
                    nc.vector.scalar_tensor_tensor(q_t[:], s_t[:], K18, dst[:],
                                                   OP.mult, OP.add)
                    nc.vector.tensor_scalar(r_t[:], q_t[:], dd18[:, j:j + 1], 0.0,
                                            OP.add, OP.add)
                    nc.vector.tensor_scalar(dst[:], r_t[:], MASK18, MASK18,
                                            OP.bitwise_and, OP.bitwise_and)
                # final partial step -> map values zL (turns, mod 1)
                nc.scalar.activation(s_t[:], dst[:], AF.Sin,
                                     bias=mpi_t[:], scale=C18)
                nc.vector.scalar_tensor_tensor(q_t[:], s_t[:], K18, dst[:],
                                               OP.mult, OP.add)
                nc.vector.tensor_scalar(zL[:], q_t[:], float(-(2.0 ** -18)),
                                        oR[:, L - 1:L], OP.mult, OP.add)

            # ---------------- unwrap maps + psi ----------------
            with nc.named_scope("maps"):
                nc.vector.tensor_tensor(wdt[:], zL[:, 1:G], zL[:, 0:G - 1],
                                        OP.subtract)
                nc.vector.tensor_scalar(rwd[:], wdt[:], 1.0, 0.0, OP.mult, OP.add)
                nc.vector.scalar_tensor_tensor(wdt[:], rwd[:], -1.0, wdt[:],
                                               OP.mult, OP.add)
                nc.vector.tensor_copy(zu[:, 0:1], zL[:, 0:1])
                nc.vector.tensor_tensor_scan(zu[:, 1:G], ones_t[:], wdt[:],
                                             zL[:, 0:1], OP.mult, OP.add)
                nc.vector.scalar_tensor_tensor(psig[:, 0:G], zu[:], GF, iotaV[:],
                                               OP.mult, OP.subtract)
                # extension column (grid point +64)
                nc.vector.tensor_tensor(w1e[:], zL[:, 0:1], zL[:, G - 1:G],
                                        OP.subtract)
                nc.vector.tensor_scalar(rex[:], w1e[:], 1.0, 0.0, OP.mult, OP.add)
                nc.vector.scalar_tensor_tensor(w2e[:], rex[:], -1.0, w1e[:],
                                               OP.mult, OP.add)
                nc.vector.tensor_tensor(zue[:], zu[:, G - 1:G], w2e[:], OP.add)
                nc.vector.tensor_scalar(psig[:, G:G + 1], zue[:], GF, 64.0,
                                        OP.mult, OP.subtract)
                qs = (nc.sync, nc.scalar, nc.gpsimd)
                for c in range(M):
                    qs[c % 3].dma_start(zmapS[:, c * (G + 1):(c + 1) * (G + 1)],
                                        psig[c * 8:(c + 1) * 8, :])

            # ---------------- stitch ----------------
            with nc.named_scope("stitch"):
                for c in range(M):
                    xin = xst[c % 2][:]
                    xout = xst[(c + 1) % 2][:]
                    nc.vector.tensor_scalar(rg_t[:], xin, float(1.0 / G), 0.0,
                                            OP.mult, OP.add)
                    nc.vector.scalar_tensor_tensor(xf_t[:], rg_t[:], -GF, xin,
                                                   OP.mult, OP.add)
                    nc.scalar.activation(dab_t[:], iotaU[:], AF.Abs,
                                         bias=xf_t[:], scale=-1.0)
                    nc.scalar.activation(wt_t[:], dab_t[:], AF.Relu,
                                         bias=one8[:], scale=-1.0)
                    nc.vector.scalar_tensor_tensor(
                        scr_t[:], wt_t[:], 1.0,
                        zmapS[:, c * (G + 1):(c + 1) * (G + 1)],
                        OP.mult, OP.mult, accum_out=y_t[:])
                    nc.vector.tensor_tensor(xout, y_t[:], xin, OP.add)

            # ---------------- tail: generation ----------------
            with nc.named_scope("tail"):
                xfin = xst[M % 2][:]
                nc.vector.tensor_scalar(zt_t[:], xfin, float(1.0 / G), 0.0,
                                        OP.mult, OP.add)
                nc.vector.tensor_scalar(rT_t[:], zt_t[:], 1.0, 0.0, OP.mult, OP.add)
                nc.vector.scalar_tensor_tensor(zw_t[:], rT_t[:], -1.0, zt_t[:],
                                               OP.mult, OP.add)
                nc.scalar.activation(az_t[:], zw_t[:], AF.Abs, bias=0.0, scale=1.0)
                nc.scalar.activation(uc_t[:], az_t[:], AF.Sin,
                                     bias=cpih[:], scale=-TWO_PI)   # cos(2pi z)
                nc.scalar.activation(vs_t[:], zw_t[:], AF.Sin,
                                     bias=0.0, scale=TWO_PI)        # sin(2pi z)
                nc.scalar.activation(r_u[:], dwsb[:], AF.Copy,
                                     bias=0.0, scale=uc_t[:])
                nc.vector.tensor_scalar(r_v[:], dwsb[:], vs_t[:], 0.0,
                                        OP.mult, OP.add)
                for half in range(2):
                    hs = slice(half * 512, (half + 1) * 512)
                    po = ps.tile([128, 512], F32, tag="po")
                    nc.tensor.matmul(po[:], csu[:], r_u[:, hs],
                                     start=True, stop=False)
                    nc.tensor.matmul(po[:], csv[:], r_v[:, hs],
                                     start=False, stop=False)
                    nc.tensor.matmul(po[:], cs1[:], dbsb[:, hs],
                                     start=False, stop=True)
                    nc.vector.tensor_copy(outsb[:, hs], po[:])
                    (nc.sync if half == 0 else nc.scalar).dma_start(
                        out[0:128, hs], outsb[:, hs])

    return nc


def kernel(**inputs) -> np.ndarray:
    _install_birfix()
    from concourse.bass_utils import run_bass_kernel_spmd
    import ml_dtypes

    bf16 = ml_dtypes.bfloat16

    X = np.asarray(inputs["observed_trajectory"], dtype=np.float32)
    W1 = np.asarray(inputs["W1"], dtype=np.float32)
    b1 = np.asarray(inputs["b1"], dtype=np.float32)
    W2 = np.asarray(inputs["W2"], dtype=np.float32)
    b2 = np.asarray(inputs["b2"], dtype=np.float64)
    freqs = np.asarray(inputs["freqs"], dtype=np.float64)
    dec_W = np.asarray(inputs["dec_W"], dtype=np.float32)
    dec_b = np.asarray(inputs["dec_b"], dtype=np.float32)
    num_steps = int(np.asarray(inputs["num_steps"]))
    T, D_ = X.shape
    assert (T, D_, num_steps) == (T_FULL, D, S_OUT), (T, D_, num_steps)

    base = freqs * TWO_PI * DT            # (A,) rad/step
    baseT = base / TWO_PI                 # turns/step

    t0 = T - W
    Xw = X[t0:]                            # (W, D)
    xT = np.ascontiguousarray(Xw.T)        # (D, W) fp32
    xhi = xT.astype(bf16)
    xlo = (xT - xhi.astype(np.float32)).astype(bf16)
    w1T = np.ascontiguousarray(W1.T)       # (D, D)
    w1hi = w1T.astype(bf16)
    w1lo = (w1T - w1hi.astype(np.float32)).astype(bf16)
    w2T = np.ascontiguousarray(W2.T)       # (D, A)
    w2hi = w2T.astype(bf16)
    w2lo = (w2T - w2hi.astype(np.float32)).astype(bf16)
    b1g = np.ascontiguousarray(b1.reshape(8, 128).T)

    j = np.arange(W, dtype=np.float64)
    ob = b2[:, None] / TWO_PI - j[None, :] * baseT[:, None]
    obT = (ob - np.round(ob)).astype(np.float32)        # (A, W) wrapped turns

    gv = np.arange(G, dtype=np.float32) - 64.0
    iotaZ0 = np.broadcast_to(gv / G, (128, G)).astype(np.float32).copy()
    iotaV = np.broadcast_to(gv, (128, G)).astype(np.float32).copy()
    iotaU = np.broadcast_to(np.arange(G + 1, dtype=np.float32) - 64.0,
                            (A, G + 1)).astype(np.float32).copy()

    decwt = np.ascontiguousarray(dec_W.T)
    decb = np.ascontiguousarray(dec_b.reshape(1, D))

    in_maps = []
    rows = S_OUT // NCORES
    for c in range(NCORES):
        s = np.arange(c * rows, (c + 1) * rows, dtype=np.float64)
        th = TWO_PI * ((W + s[None, :] + 1.0) * baseT[:, None])   # (A, rows)
        csm = np.empty((17, rows), np.float32)
        csm[0:8] = np.cos(th)
        csm[8:16] = -np.sin(th)
        csm[16] = 1.0
        in_maps.append({
            "xhiT": xhi, "xloT": np.ascontiguousarray(xlo[:, SPLIT:]),
            "w1hiT": w1hi, "w1loT": w1lo,
            "w2hiT": w2hi, "w2loT": w2lo,
            "b1g": b1g, "obT": obT,
            "iotaZ0": iotaZ0, "iotaV": iotaV, "iotaU": iotaU,
            "decwt": decwt, "decb": decb,
            "cs": np.ascontiguousarray(csm),
        })

    if "nc" not in _cache:
        _cache["nc"] = _build_nc()
    res = run_bass_kernel_spmd(_cache["nc"], in_maps, core_ids=list(range(NCORES)))
    out = np.concatenate([r["out"] for r in res.results], axis=0)
    return out.astype(np.float32)


# revision 21
# speedup vs baseline: 1.2434x; 1.0104x over previous
"""Trainium2 Bass kernel for nn_MirrorResonance.

Math summary
------------
reference: H = tanh(X @ W1.T + b1); E = H @ W2.T + b2; o = wrap(E)
           phases: p <- mod(p + base + K*sin(o_t - p), 2pi)  over T=16384 steps
           out[s] = cos(phases + (s+1)*base) @ dec_W.T + dec_b

Design (all phase math in fp32 TURN units, z-transformed z_j = p_j - j*base):
 * Truncation: the scan contracts (~0.971/step); only the last W=576 steps
   matter (validated: 2.4e-4 rel err of exact truncated scan).
 * Encoder: hybrid precision. X/W1 split into bf16 hi+lo; early window
   columns (noise contracted away) use 1 product (hi*hi), late columns use 3
   (hi*hi + lo*hi + hi*lo) which restores ~fp32 accuracy. Same for H@W2.
 * Parallel-in-time scan: split W into M=16 chunks of L=36 steps, one
   partition group (8 attractors) per chunk, and run each chunk's scan from
   G=128 grid initial conditions living in the free axis. All chunks advance
   in lockstep: L serial steps instead of W.
   Step (d-state = wrapped angle diff, turns): s=sin(2pi d);
   q = d - (K/2pi)s; d' = wrap(q + delta_j) via round-to-int32 trick.
 * Each chunk yields its end-map F_c on the grid. Maps are unwrapped along
   the grid axis with one hardware scan instruction (cumsum of wrapped
   neighbor diffs), converted to deviation-from-identity psi in grid units.
 * Stitch: 16 sequential tent-weight interpolations (|.|/relu/fused
   multiply-reduce) compose the chunk maps at the running boundary estimate.
 * Generation: rank-17 matmul with host-precomputed cos/sin(t*base) rows,
   sharded over the 8 cores (128 output rows each).

Full-pipeline numpy simulation of this exact algorithm: rel err 8.8e-4.
"""

import numpy as np

TWO_PI = 2.0 * np.pi
DT = 0.01
K = 0.5
NCORES = 8
T_FULL = 16384
D = 1024
A = 8
S_OUT = 1024

W = 576          # scan window
M = 16           # chunks
L = W // M       # serial steps (36)
G = 128          # grid points per chunk map
SPLIT = 396      # window column where high-precision encoder starts
CH = 396         # encoder column chunk (psum-bank sized)

_cache = {}


# ---------------------------------------------------------------------------
# BIR legalization: this walrus build supports at most ONE sync-wait per
# instruction; split extra waits into single-wait EventSemaphore predecessors
# on the same engine (semantics preserved: engine stalls before the op).
# ---------------------------------------------------------------------------
def _install_birfix():
    if _cache.get("birfix"):
        return
    import orjson
    import concourse.bass_utils as bu
    import concourse.bass2jax as b2j

    orig = bu.compile_bir_kernel

    def _legalize(bir: bytes) -> bytes:
        d = orjson.loads(bir)
        for fn in d.get("functions", []):
            for blk in fn.get("blocks", []):
                out = []
                for inst in blk.get("instructions", []):
                    si = inst.get("sync_info") or {}
                    waits = si.get("on_wait") or []
                    if len(waits) > 1:
                        for k, w in enumerate(waits[:-1]):
                            out.append({
                                "debug": inst.get("debug", 0),
                                "engine": inst["engine"],
                                "ins": [], "outs": [],
                                "name": f"{inst['name']}_w{k}",
                                "opcode": "EventSemaphore",
                                "sync_info": {"on_update": [], "on_wait": [w]},
                            })
                        si["on_wait"] = [waits[-1]]
                    out.append(inst)
                blk["instructions"] = out
        return orjson.dumps(d)

    def wrapped(bir_json: bytes, tmpdir: str, neff_name="file.neff"):
        return orig(_legalize(bir_json), tmpdir, neff_name)

    bu.compile_bir_kernel = wrapped
    b2j.compile_bir_kernel = wrapped
    _cache["birfix"] = True


def _build_nc():
    import concourse.bass as bass
    import concourse.tile as tile
    import concourse.mybir as mybir
    from concourse.alu_op_type import AluOpType as OP

    F32 = mybir.dt.float32
    I32 = mybir.dt.int32
    BF16 = mybir.dt.bfloat16
    AF = mybir.ActivationFunctionType
    INV_2PI = float(1.0 / TWO_PI)
    KT = float(K / TWO_PI)
    SC18 = float(2 ** 18)
    C18 = float(TWO_PI / 2 ** 18)
    K18 = float(K / TWO_PI * 2 ** 18)
    MASK18 = (1 << 18) - 1
    NEG_PI = float(-np.pi)
    HALF_PI = float(np.pi / 2.0)
    GF = float(G)
    LATE = W - SPLIT  # columns in the high-precision range

    nc = bass.Bass("TRN2")
    xhiT = nc.dram_tensor("xhiT", [D, W], BF16, kind="ExternalInput")
    xloT = nc.dram_tensor("xloT", [D, LATE], BF16, kind="ExternalInput")
    w1hiT = nc.dram_tensor("w1hiT", [D, D], BF16, kind="ExternalInput")
    w1loT = nc.dram_tensor("w1loT", [D, D], BF16, kind="ExternalInput")
    w2hiT = nc.dram_tensor("w2hiT", [D, A], BF16, kind="ExternalInput")
    w2loT = nc.dram_tensor("w2loT", [D, A], BF16, kind="ExternalInput")
    b1g = nc.dram_tensor("b1g", [128, 8], F32, kind="ExternalInput")
    obT_d = nc.dram_tensor("obT", [A, W], F32, kind="ExternalInput")
    iotaZ0_d = nc.dram_tensor("iotaZ0", [128, G], F32, kind="ExternalInput")
    iotaV_d = nc.dram_tensor("iotaV", [128, G], F32, kind="ExternalInput")
    iotaU_d = nc.dram_tensor("iotaU", [A, G + 1], F32, kind="ExternalInput")
    decwt = nc.dram_tensor("decwt", [A, D], F32, kind="ExternalInput")
    decb = nc.dram_tensor("decb", [1, D], F32, kind="ExternalInput")
    cs = nc.dram_tensor("cs", [17, 128], F32, kind="ExternalInput")
    out = nc.dram_tensor("out", [128, D], F32, kind="ExternalOutput")

    with tile.TileContext(nc) as tc:
        with (
            tc.tile_pool(name="sb", bufs=1) as sb,
            tc.tile_pool(name="ps", bufs=2, space="PSUM") as ps,
            tc.tile_pool(name="ps1", bufs=1, space="PSUM") as ps1,
        ):
            PC = SPLIT // L          # first high-precision chunk (11)
            NP = M - PC              # number of precise chunks (5)
            w1h = [sb.tile([128, D], BF16, name=f"w1h{k}", tag=f"w1h{k}") for k in range(8)]
            w1l = [sb.tile([128, D], BF16, name=f"w1l{k}", tag=f"w1l{k}") for k in range(8)]
            xh = [sb.tile([128, M, L], BF16, name=f"xh{k}", tag=f"xh{k}") for k in range(8)]
            xl = [sb.tile([128, NP, L], BF16, name=f"xl{k}", tag=f"xl{k}") for k in range(8)]
            w2h = sb.tile([128, 8, A], BF16)
            w2l = sb.tile([128, 8, A], BF16)
            b1sb = sb.tile([128, 8], F32)
            obT = sb.tile([A, M, L], F32)
            hf = sb.tile([128, NP, L // 2], F32)
            hhi = [sb.tile([128, M, L], BF16, name=f"hhi{n}", tag=f"hhi{n}") for n in range(8)]
            hlo = [sb.tile([128, NP, L], BF16, name=f"hlo{n}", tag=f"hlo{n}") for n in range(8)]
            e23 = sb.tile([A, M, L], F32)
            rE3 = sb.tile([A, M, L], I32)
            oTt3 = sb.tile([A, M, L], F32)
            oR = sb.tile([128, L], F32)
            oR18 = sb.tile([128, L], F32)
            dd18 = sb.tile([128, L - 1], F32)
            mpi_t = sb.tile([128, 1], F32)
            iotaZ0 = sb.tile([128, G], F32)
            iotaV = sb.tile([128, G], F32)
            iotaU = sb.tile([A, G + 1], F32)
            dst = sb.tile([128, G], I32)
            s_t = sb.tile([128, G], F32)
            q_t = sb.tile([128, G], F32)
            r_t = sb.tile([128, G], I32)
            zL = sb.tile([128, G], F32)
            wdt = sb.tile([128, G - 1], F32)
            rwd = sb.tile([128, G - 1], I32)
            ones_t = sb.tile([128, G - 1], F32)
            zu = sb.tile([128, G], F32)
            psig = sb.tile([128, G + 1], F32)
            w1e = sb.tile([128, 1], F32)
            rex = sb.tile([128, 1], I32)
            w2e = sb.tile([128, 1], F32)
            zue = sb.tile([128, 1], F32)
            zmapS = sb.tile([A, M * (G + 1)], F32)
            xst = [sb.tile([A, 1], F32, name=f"xst{i}") for i in range(2)]
            rg_t = sb.tile([A, 1], I32)
            xf_t = sb.tile([A, 1], F32)
            dab_t = sb.tile([A, G + 1], F32)
            wt_t = sb.tile([A, G + 1], F32)
            scr_t = sb.tile([A, G + 1], F32)
            y_t = sb.tile([A, 1], F32)
            one8 = sb.tile([A, 1], F32)
            cpih = sb.tile([A, 1], F32)
            zt_t = sb.tile([A, 1], F32)
            rT_t = sb.tile([A, 1], I32)
            zw_t = sb.tile([A, 1], F32)
            az_t = sb.tile([A, 1], F32)
            uc_t = sb.tile([A, 1], F32)
            vs_t = sb.tile([A, 1], F32)
            dwsb = sb.tile([A, D], F32)
            r_u = sb.tile([A, D], F32)
            r_v = sb.tile([A, D], F32)
            dbsb = sb.tile([1, D], F32)
            csu = sb.tile([A, 128], F32)
            csv = sb.tile([A, 128], F32)
            cs1 = sb.tile([1, 128], F32)
            outsb = sb.tile([128, D], F32)

            dma = nc.sync
            dq = (nc.sync, nc.scalar, nc.gpsimd)
            for k in range(8):
                dq[k % 3].dma_start(xh[k][:], xhiT[k * 128:(k + 1) * 128, :])
                dq[(k + 1) % 3].dma_start(w1h[k][:], w1hiT[k * 128:(k + 1) * 128, :])
            for k in range(8):
                dq[k % 3].dma_start(xl[k][:], xloT[k * 128:(k + 1) * 128, :])
                dq[(k + 1) % 3].dma_start(w1l[k][:], w1loT[k * 128:(k + 1) * 128, :])
                dq[(k + 2) % 3].dma_start(w2h[:, k, :], w2hiT[k * 128:(k + 1) * 128, :])
                dq[k % 3].dma_start(w2l[:, k, :], w2loT[k * 128:(k + 1) * 128, :])
            dma.dma_start(b1sb[:], b1g[:])
            dma.dma_start(obT[:], obT_d[:])
            HB = L // 2   # scan steps per encoder block (18)
            dma.dma_start(iotaZ0[:], iotaZ0_d[:])
            dma.dma_start(iotaV[:], iotaV_d[:])
            dma.dma_start(iotaU[:], iotaU_d[:])
            dma.dma_start(dwsb[:], decwt[:])
            dma.dma_start(dbsb[:], decb[:])
            dma.dma_start(csu[:], cs[0:8, :])
            dma.dma_start(csv[:], cs[8:16, :])
            dma.dma_start(cs1[:], cs[16:17, :])
            nc.vector.memset(ones_t[:], 1.0)
            nc.vector.memset(one8[:], 1.0)
            nc.vector.memset(cpih[:], HALF_PI)
            nc.vector.memset(xst[0][:], 0.0)
            nc.vector.memset(mpi_t[:], NEG_PI)

            # ------- encoder, in two scan-step blocks so the scan overlaps -------
            with nc.named_scope("encoder"):
                qs = (nc.sync, nc.scalar, nc.gpsimd)
                for b in range(2):
                    isl = slice(b * HB, (b + 1) * HB)
                    pec = ps1.tile([A, PC, HB], F32, tag="pec")
                    pep = ps1.tile([A, NP, HB], F32, tag="pep")

                    def emit_pe(nt):
                        nc.tensor.matmul(pec[:], w2h[:, nt, :],
                                         hhi[nt][:, 0:PC, isl],
                                         start=(nt == 0), stop=(nt == 7))
                        nc.tensor.matmul(pep[:], w2h[:, nt, :],
                                         hhi[nt][:, PC:M, isl],
                                         start=(nt == 0), stop=False)
                        nc.tensor.matmul(pep[:], w2h[:, nt, :],
                                         hlo[nt][:, :, isl],
                                         start=False, stop=False)
                        nc.tensor.matmul(pep[:], w2l[:, nt, :],
                                         hhi[nt][:, PC:M, isl],
                                         start=False, stop=(nt == 7))

                    for nt in range(8):
                        nsl = slice(nt * 128, (nt + 1) * 128)
                        phc = ps.tile([128, PC, HB], F32, tag="phc")
                        php = ps.tile([128, NP, HB], F32, tag="php")
                        for kt in range(8):
                            last = kt == 7
                            nc.tensor.matmul(phc[:], w1h[kt][:, nsl],
                                             xh[kt][:, 0:PC, isl],
                                             start=(kt == 0), stop=last)
                            nc.tensor.matmul(php[:], w1h[kt][:, nsl],
                                             xh[kt][:, PC:M, isl],
                                             start=(kt == 0), stop=False)
                            nc.tensor.matmul(php[:], w1h[kt][:, nsl],
                                             xl[kt][:, :, isl],
                                             start=False, stop=False)
                            nc.tensor.matmul(php[:], w1l[kt][:, nsl],
                                             xh[kt][:, PC:M, isl],
                                             start=False, stop=last)
                        nc.scalar.activation(hhi[nt][:, 0:PC, isl], phc[:],
                                             AF.Tanh,
                                             bias=b1sb[:, nt:nt + 1], scale=1.0)
                        nc.scalar.activation(hf[:], php[:], AF.Tanh,
                                             bias=b1sb[:, nt:nt + 1], scale=1.0)
                        nc.vector.tensor_scalar(hhi[nt][:, PC:M, isl], hf[:],
                                                1.0, 0.0, OP.mult, OP.add)
                        nc.vector.scalar_tensor_tensor(hlo[nt][:, :, isl],
                                                       hhi[nt][:, PC:M, isl],
                                                       -1.0, hf[:],
                                                       OP.mult, OP.add)
                        if nt >= 1:
                            emit_pe(nt - 1)
                    emit_pe(7)
                    # e2 = E/2pi + obT (turns), wrap, rearrange, deltas
                    nc.vector.scalar_tensor_tensor(e23[:, 0:PC, isl], pec[:],
                                                   INV_2PI, obT[:, 0:PC, isl],
                                                   OP.mult, OP.add)
                    nc.vector.scalar_tensor_tensor(e23[:, PC:M, isl], pep[:],
                                                   INV_2PI, obT[:, PC:M, isl],
                                                   OP.mult, OP.add)
                    nc.vector.tensor_scalar(rE3[:, :, isl], e23[:, :, isl],
                                            1.0, 0.0, OP.mult, OP.add)
                    nc.vector.scalar_tensor_tensor(oTt3[:, :, isl], rE3[:, :, isl],
                                                   -1.0, e23[:, :, isl],
                                                   OP.mult, OP.add)
                    for c in range(M):
                        qs[c % 3].dma_start(oR[c * 8:(c + 1) * 8, isl],
                                            oTt3[:, c, isl])
                    nc.vector.tensor_scalar(oR18[:, isl], oR[:, isl], SC18, 0.0,
                                            OP.mult, OP.add)
                    dsl = slice(b * HB - (1 if b else 0), (b + 1) * HB - 1)
                    nc.vector.tensor_tensor(dd18[:, dsl],
                                            oR18[:, dsl.start + 1:dsl.stop + 1],
                                            oR18[:, dsl.start:dsl.stop], OP.subtract)

            # ------- grid scan: int32 phase state, AND-mod range reduction ----
            # state dst = d mod 1 turn, in 2^-18-turn units.  Sin(C18*dst - pi)
            # = -sin(2pi d): the sign is absorbed into +K18 below.
            with nc.named_scope("scan"):
                nc.vector.tensor_scalar(q_t[:], iotaZ0[:], -SC18, oR18[:, 0:1],
                                        OP.mult, OP.add)
                nc.vector.tensor_scalar(r_t[:], q_t[:], 1.0, 0.0, OP.mult, OP.add)
                nc.vector.tensor_scalar(dst[:], r_t[:], MASK18, MASK18,
                                        OP.bitwise_and, OP.bitwise_and)
                for j in range(L - 1):
                    nc.scalar.activation(s_t[:], dst[:], AF.Sin,
                                         bias=mpi_t[:], scale=C18)
                    nc.vector.scalar_tensor_tensor(q_t[:], s_t[:], K18, dst[:],
                                                   OP.mult, OP.add)
                    nc.vector.tensor_scalar(r_t[:], q_t[:], dd18[:, j:j + 1], 0.0,
                                            OP.add, OP.add)
                    nc.vector.tensor_scalar(dst[:], r_t[:], MASK18, MASK18,
                                            OP.bitwise_and, OP.bitwise_and)
                # final partial step -> map values zL (turns, mod 1)
                nc.scalar.activation(s_t[:], dst[:], AF.Sin,
                                     bias=mpi_t[:], scale=C18)
                nc.vector.scalar_tensor_tensor(q_t[:], s_t[:], K18, dst[:],
                                               OP.mult, OP.add)
                nc.vector.tensor_scalar(zL[:], q_t[:], float(-(2.0 ** -18)),
                                        oR[:, L - 1:L], OP.mult, OP.add)

            # ---------------- unwrap maps + psi ----------------
            with nc.named_scope("maps"):
                nc.vector.tensor_tensor(wdt[:], zL[:, 1:G], zL[:, 0:G - 1],
                                        OP.subtract)
                nc.vector.tensor_scalar(rwd[:], wdt[:], 1.0, 0.0, OP.mult, OP.add)
                nc.vector.scalar_tensor_tensor(wdt[:], rwd[:], -1.0, wdt[:],
                                               OP.mult, OP.add)
                nc.vector.tensor_copy(zu[:, 0:1], zL[:, 0:1])
                nc.vector.tensor_tensor_scan(zu[:, 1:G], ones_t[:], wdt[:],
                                             zL[:, 0:1], OP.mult, OP.add)
                nc.vector.scalar_tensor_tensor(psig[:, 0:G], zu[:], GF, iotaV[:],
                                               OP.mult, OP.subtract)
                # extension column (grid point +64)
                nc.vector.tensor_tensor(w1e[:], zL[:, 0:1], zL[:, G - 1:G],
                                        OP.subtract)
                nc.vector.tensor_scalar(rex[:], w1e[:], 1.0, 0.0, OP.mult, OP.add)
                nc.vector.scalar_tensor_tensor(w2e[:], rex[:], -1.0, w1e[:],
                                               OP.mult, OP.add)
                nc.vector.tensor_tensor(zue[:], zu[:, G - 1:G], w2e[:], OP.add)
                nc.vector.tensor_scalar(psig[:, G:G + 1], zue[:], GF, 64.0,
                                        OP.mult, OP.subtract)
                qs = (nc.sync, nc.scalar, nc.gpsimd)
                for c in range(M):
                    qs[c % 3].dma_start(zmapS[:, c * (G + 1):(c + 1) * (G + 1)],
                                        psig[c * 8:(c + 1) * 8, :])

            # ---------------- stitch ----------------
            with nc.named_scope("stitch"):
                for c in range(M):
                    xin = xst[c % 2][:]
                    xout = xst[(c + 1) % 2][:]
                    nc.vector.tensor_scalar(rg_t[:], xin, float(1.0 / G), 0.0,
                                            OP.mult, OP.add)
                    nc.vector.scalar_tensor_tensor(xf_t[:], rg_t[:], -GF, xin,
                                                   OP.mult, OP.add)
                    nc.scalar.activation(dab_t[:], iotaU[:], AF.Abs,
                                         bias=xf_t[:], scale=-1.0)
                    nc.scalar.activation(wt_t[:], dab_t[:], AF.Relu,
                                         bias=one8[:], scale=-1.0)
                    nc.vector.scalar_tensor_tensor(
                        scr_t[:], wt_t[:], 1.0,
                        zmapS[:, c * (G + 1):(c + 1) * (G + 1)],
                        OP.mult, OP.mult, accum_out=y_t[:])
                    nc.vector.tensor_tensor(xout, y_t[:], xin, OP.add)

            # ---------------- tail: generation ----------------
            with nc.named_scope("tail"):
                xfin = xst[M % 2][:]
                nc.vector.tensor_scalar(zt_t[:], xfin, float(1.0 / G), 0.0,
                                        OP.mult, OP.add)
                nc.vector.tensor_scalar(rT_t[:], zt_t[:], 1.0, 0.0, OP.mult, OP.add)
                nc.vector.scalar_tensor_tensor(zw_t[:], rT_t[:], -1.0, zt_t[:],
                                               OP.mult, OP.add)
                nc.scalar.activation(az_t[:], zw_t[:], AF.Abs, bias=0.0, scale=1.0)
                nc.scalar.activation(uc_t[:], az_t[:], AF.Sin,
                                     bias=cpih[:], scale=-TWO_PI)   # cos(2pi z)
                nc.scalar.activation(vs_t[:], zw_t[:], AF.Sin,
                                     bias=0.0, scale=TWO_PI)        # sin(2pi z)
                nc.scalar.activation(r_u[:], dwsb[:], AF.Copy,
                                     bias=0.0, scale=uc_t[:])
                nc.vector.tensor_scalar(r_v[:], dwsb[:], vs_t[:], 0.0,
                                        OP.mult, OP.add)
                for half in range(2):
                    hs = slice(half * 512, (half + 1) * 512)
                    po = ps.tile([128, 512], F32, tag="po")
                    nc.tensor.matmul(po[:], csu[:], r_u[:, hs],
                                     start=True, stop=False)
                    nc.tensor.matmul(po[:], csv[:], r_v[:, hs],
                                     start=False, stop=False)
                    nc.tensor.matmul(po[:], cs1[:], dbsb[:, hs],
                                     start=False, stop=True)
                    nc.vector.tensor_copy(outsb[:, hs], po[:])
                    (nc.sync if half == 0 else nc.scalar).dma_start(
                        out[0:128, hs], outsb[:, hs])

    return nc


def kernel(**inputs) -> np.ndarray:
    _install_birfix()
    from concourse.bass_utils import run_bass_kernel_spmd
    import ml_dtypes

    bf16 = ml_dtypes.bfloat16

    X = np.asarray(inputs["observed_trajectory"], dtype=np.float32)
    W1 = np.asarray(inputs["W1"], dtype=np.float32)
    b1 = np.asarray(inputs["b1"], dtype=np.float32)
    W2 = np.asarray(inputs["W2"], dtype=np.float32)
    b2 = np.asarray(inputs["b2"], dtype=np.float64)
    freqs = np.asarray(inputs["freqs"], dtype=np.float64)
    dec_W = np.asarray(inputs["dec_W"], dtype=np.float32)
    dec_b = np.asarray(inputs["dec_b"], dtype=np.float32)
    num_steps = int(np.asarray(inputs["num_steps"]))
    T, D_ = X.shape
    assert (T, D_, num_steps) == (T_FULL, D, S_OUT), (T, D_, num_steps)

    base = freqs * TWO_PI * DT            # (A,) rad/step
    baseT = base / TWO_PI                 # turns/step

    t0 = T - W
    Xw = X[t0:]                            # (W, D)
    xT = np.ascontiguousarray(Xw.T)        # (D, W) fp32
    xhi = xT.astype(bf16)
    xlo = (xT - xhi.astype(np.float32)).astype(bf16)
    w1T = np.ascontiguousarray(W1.T)       # (D, D)
    w1hi = w1T.astype(bf16)
    w1lo = (w1T - w1hi.astype(np.float32)).astype(bf16)
    w2T = np.ascontiguousarray(W2.T)       # (D, A)
    w2hi = w2T.astype(bf16)
    w2lo = (w2T - w2hi.astype(np.float32)).astype(bf16)
    b1g = np.ascontiguousarray(b1.reshape(8, 128).T)

    j = np.arange(W, dtype=np.float64)
    ob = b2[:, None] / TWO_PI - j[None, :] * baseT[:, None]
    obT = (ob - np.round(ob)).astype(np.float32)        # (A, W) wrapped turns

    gv = np.arange(G, dtype=np.float32) - 64.0
    iotaZ0 = np.broadcast_to(gv / G, (128, G)).astype(np.float32).copy()
    iotaV = np.broadcast_to(gv, (128, G)).astype(np.float32).copy()
    iotaU = np.broadcast_to(np.arange(G + 1, dtype=np.float32) - 64.0,
                            (A, G + 1)).astype(np.float32).copy()

    decwt = np.ascontiguousarray(dec_W.T)
    decb = np.ascontiguousarray(dec_b.reshape(1, D))

    in_maps = []
    rows = S_OUT // NCORES
    for c in range(NCORES):
        s = np.arange(c * rows, (c + 1) * rows, dtype=np.float64)
        th = TWO_PI * ((W + s[None, :] + 1.0) * baseT[:, None])   # (A, rows)
        csm = np.empty((17, rows), np.float32)
        csm[0:8] = np.cos(th)
        csm[8:16] = -np.sin(th)
        csm[16] = 1.0
        in_maps.append({
            "xhiT": xhi, "xloT": np.ascontiguousarray(xlo[:, SPLIT:]),
            "w1hiT": w1hi, "w1loT": w1lo,
            "w2hiT": w2hi, "w2loT": w2lo,
            "b1g": b1g, "obT": obT,
            "iotaZ0": iotaZ0, "iotaV": iotaV, "iotaU": iotaU,
            "decwt": decwt, "decb": decb,
            "cs": np.ascontiguousarray(csm),
        })

    if "nc" not in _cache:
        _cache["nc"] = _build_nc()
    res = run_bass_kernel_spmd(_cache["nc"], in_maps, core_ids=list(range(NCORES)))
    out = np.concatenate([r["out"] for r in res.results], axis=0)
    return out.astype(np.float32)
